# revision 12
# baseline (speedup 1.0000x reference)
"""Block-sparse linear kernel for Trainium2 (8 NeuronCores, Bass/Tile).

Computes out[n, ob*BS:(ob+1)*BS] += x[n, ib*BS:(ib+1)*BS] @ W[k]  for each
nonzero block k with indices (ob, ib), plus bias — data-parallel over the
flattened row dim N across 8 cores (weights/indices replicated).

Strategy (host-side schedule specialization from the index tensors):
  - Group input-blocks (ibs) into *families* with identical sets of
    output-blocks (obs).  Families whose obs-sets overlap are merged into
    *superfamilies* (zero-filled weight stacking keeps those correct).
  - Within a family, pair up ibs: a pair forms one K=128 stationary
    operand (the pair's two 64-feature slices of x, transposed host-side),
    streaming a [128, n_obs*64] stacked weight -> full PE utilization.
  - PSUM accumulates each superfamily-segment (<=16 obs = <=1024 f32 cols
    = 2 banks) over all its pairs/singles via matmul start/stop flags.
  - Output is laid out family-major (contiguous evictions); the host
    un-permutes output block columns and adds bias.
  - Matmuls run in float32r (TF32-like, ~1.5e-4 rel err, full PE rate).

The device kernel does: 2 input DMAs, matmul accumulation, PSUM->SBUF
evictions on ACT/DVE, 1 output DMA per 128-row tile.  All x transposition
and index logic happens on the host at schedule-build time.
"""

import numpy as np
from collections import defaultdict
from contextlib import ExitStack

from concourse import bass_utils, bacc, mybir
import concourse.tile as tile

N_CORES = 8
P = 128            # partitions / row-tile size
SEG_MAX_OBS = 16   # psum segment cap: 16 blocks * 64 = 1024 f32 = 2 banks
F32R = mybir.dt.float32r
F32 = mybir.dt.float32

# schedule-key -> (nc, meta) cache so repeated kernel() calls reuse the
# compiled module (and the NEFF cache underneath).
_CACHE = {}

# test harness introspection: last BassKernelResults
LAST_RESULT = None


def _build_schedule(N, F, OUT_F, BS, out_idx, in_idx):
    """Pure-index schedule: families, pairs, segments, layouts."""
    n_ib = F // BS
    n_ob = OUT_F // BS
    assert F % BS == 0 and OUT_F % BS == 0

    # (ob, ib) -> list of weight slots k (duplicates summed host-side)
    wslots = defaultdict(list)
    for k, (ob, ib) in enumerate(zip(out_idx, in_idx)):
        ob, ib = int(ob), int(ib)
        assert 0 <= ob < n_ob and 0 <= ib < n_ib
        wslots[(ob, ib)].append(k)

    obs_by_ib = defaultdict(set)
    for (ob, ib) in wslots:
        obs_by_ib[ib].add(ob)

    # families: ibs with identical obs sets
    fam_map = defaultdict(list)
    for ib in sorted(obs_by_ib):
        fam_map[frozenset(obs_by_ib[ib])].append(ib)
    families = [(sorted(obs), ibs) for obs, ibs in fam_map.items()]

    # union-find over obs to merge overlapping families into superfamilies
    parent = {}

    def find(a):
        while parent[a] != a:
            parent[a] = parent[parent[a]]
            a = parent[a]
        return a

    for obs, _ in families:
        for ob in obs:
            parent.setdefault(ob, ob)
        r = find(obs[0])
        for ob in obs[1:]:
            parent[find(ob)] = r
    sf_map = defaultdict(lambda: {"obs": set(), "fams": []})
    for obs, ibs in families:
        root = find(obs[0])
        sf_map[root]["obs"].update(obs)
        sf_map[root]["fams"].append((obs, ibs))
    superfams = sorted(sf_map.values(), key=lambda s: min(s["obs"]))

    # xt tile table: pairs (full K=128) and packed singles (K=64 halves)
    xt_tiles = []      # per tile: list of (rowbase, ib) entries
    unit_of = {}       # (fam_id, pair_idx) -> (tile_idx, rowbase, krows, ibs)
    singles = []       # deferred: (fam_key, ib)
    fam_units = defaultdict(list)   # fam key -> [(tile, rowbase, krows, ibs)]
    fam_id = 0
    fam_keys = {}
    for sf in superfams:
        for obs, ibs in sf["fams"]:
            key = fam_id
            fam_keys[key] = (tuple(obs), tuple(ibs))
            for i in range(0, len(ibs) - 1, 2):
                t = len(xt_tiles)
                xt_tiles.append([(0, ibs[i]), (64, ibs[i + 1])])
                fam_units[key].append((t, 0, 128, (ibs[i], ibs[i + 1])))
            if len(ibs) % 2:
                singles.append((key, ibs[-1]))
            fam_id += 1
    for j in range(0, len(singles), 2):
        t = len(xt_tiles)
        entries = [(0, singles[j][1])]
        fam_units[singles[j][0]].append((t, 0, 64, (singles[j][1],)))
        if j + 1 < len(singles):
            entries.append((64, singles[j + 1][1]))
            fam_units[singles[j + 1][0]].append((t, 64, 64, (singles[j + 1][1],)))
        xt_tiles.append(entries)

    # segments + ws layout + out layout
    # mm task: (psum_c0, psum_c1, tile, rowbase, krows, ws_c0, start, stop)
    segments = []   # per segment: dict(out_base, n_obs, obs, tasks)
    ws_blocks = []  # (ws_col, rowbase, ib_or_None, obs_list) for host fill
    ws_cols = 0
    out_cols = 0
    fid = 0
    for sf in superfams:
        sf_obs = sorted(sf["obs"])
        # family units of this superfamily, in deterministic order
        units = []
        base = fid
        for obs, ibs in sf["fams"]:
            units.append((fid, tuple(obs)))
            fid += 1
        for s0 in range(0, len(sf_obs), SEG_MAX_OBS):
            seg_obs = sf_obs[s0:s0 + SEG_MAX_OBS]
            L = len(seg_obs) * BS
            tasks = []
            all_units = []
            for key, fobs in units:
                for (t, rb, kr, uibs) in fam_units[key]:
                    all_units.append((t, rb, kr, uibs))
            seg_ws0 = ws_cols
            unit_ws = []
            unit_tiles = []
            for ui, (t, rb, kr, uibs) in enumerate(all_units):
                wc = ws_cols
                ws_blocks.append((wc, rb, uibs, seg_obs))
                unit_ws.append((wc, wc + L))
                unit_tiles.append(t)
                for c0 in range(0, L, 512):
                    c1 = min(c0 + 512, L)
                    tasks.append((c0, c1, t, rb, kr, wc + c0,
                                  ui == 0, ui == len(all_units) - 1))
                ws_cols += L
            segments.append({"out_base": out_cols, "n_obs": len(seg_obs),
                             "obs": seg_obs, "tasks": tasks,
                             "ws0": seg_ws0, "ws1": ws_cols,
                             "unit_ws": unit_ws, "unit_tiles": unit_tiles,
                             "tiles": sorted({tk[2] for tk in tasks})})
            out_cols += L

    n_pad = (-N) % (N_CORES * P)
    rows_per_core = (N + n_pad) // N_CORES
    rt_count = rows_per_core // P

    # input-DMA load plan in consumption order: ("ws"|"xt", c0, c1).
    # xt entries are tile-index ranges; first segment's ws goes per-unit so
    # the very first matmul only waits on a small chunk.
    load_plan = []
    seen_tiles = set()

    def add_tiles(tiles):
        new_t = [t for t in tiles if t not in seen_tiles]
        seen_tiles.update(new_t)
        i = 0
        while i < len(new_t):   # merge consecutive tile indices into ranges
            j = i
            while j + 1 < len(new_t) and new_t[j + 1] == new_t[j] + 1:
                j += 1
            load_plan.append(("xt", new_t[i], new_t[j] + 1))
            i = j + 1

    for si, seg in enumerate(segments):
        if si == 0:
            # finest interleave: each unit's ws chunk followed by its xt
            # tile, so the first matmul waits on ~0.7MB only
            for (a, b), t in zip(seg["unit_ws"], seg["unit_tiles"]):
                load_plan.append(("ws", a, b))
                add_tiles([t])
        else:
            load_plan.append(("ws", seg["ws0"], seg["ws1"]))
            add_tiles(seg["tiles"])

    return {
        "N": N, "F": F, "OUT_F": OUT_F, "BS": BS,
        "wslots": dict(wslots),
        "xt_tiles": xt_tiles,
        "ws_blocks": ws_blocks, "ws_cols": ws_cols,
        "segments": segments, "out_cols": out_cols,
        "rows_per_core": rows_per_core, "rt_count": rt_count,
        "load_plan": load_plan,
    }


def _build_nc(meta):
    """Emit the Bass/Tile module for a schedule (value-independent)."""
    Nc = meta["rows_per_core"]
    XTC = len(meta["xt_tiles"]) * Nc
    WSC = meta["ws_cols"]
    OUTC = meta["out_cols"]
    rt_count = meta["rt_count"]

    nc = bacc.Bacc("TRN2", target_bir_lowering=False, debug=False)
    xt_d = nc.dram_tensor("xt", [P, XTC], F32R, kind="ExternalInput")
    ws_d = nc.dram_tensor("ws", [P, WSC], F32R, kind="ExternalInput")
    out_d = nc.dram_tensor("out", [Nc, OUTC], F32, kind="ExternalOutput")

    import os
    n_warm = int(os.environ.get("KWARM", "10"))

    with tile.TileContext(nc) as tc, ExitStack() as ctx:
        xt_pool = ctx.enter_context(tc.tile_pool(name="xt", bufs=1))
        ws_pool = ctx.enter_context(tc.tile_pool(name="ws", bufs=1))
        warm_pool = ctx.enter_context(tc.tile_pool(name="wm", bufs=1))
        psum_pool = ctx.enter_context(tc.tile_pool(name="ps", bufs=4, space="PSUM"))
        out_pool = ctx.enter_context(tc.tile_pool(name="ot", bufs=2))

        xt = xt_pool.tile([P, XTC], F32R)
        ws = ws_pool.tile([P, WSC], F32R)

        # PE warm-up: dummy matmuls with no DMA deps run during the input
        # load and flip HAM to 8/8 before the first real matmul.
        if n_warm:
            wsb = warm_pool.tile([P, 512], F32R)
            nc.vector.memset(wsb[:].bitcast(F32), 0)
            wps = psum_pool.tile([P, 1024], F32, tag="mm")
            for _ in range(n_warm):
                nc.tensor.matmul(wps[:, :512], wsb[:, :P], wsb[:, :512],
                                 start=True, stop=True)

        # chunked input DMAs in first-use order so matmuls start early
        for (kind, a, b) in meta["load_plan"]:
            if kind == "ws":
                nc.sync.dma_start(out=ws[:, a:b], in_=ws_d[:, a:b])
            else:
                nc.sync.dma_start(out=xt[:, a * Nc:b * Nc], in_=xt_d[:, a * Nc:b * Nc])

        ev = 0
        for rt in range(rt_count):
            out_sb = out_pool.tile([P, OUTC], F32)
            flushed = 0
            for si, seg in enumerate(meta["segments"]):
                L = seg["n_obs"] * meta["BS"]
                psum = psum_pool.tile([P, 1024], F32, tag="mm")
                for (c0, c1, t, rb, kr, wc, start, stop) in seg["tasks"]:
                    lhsT = xt[rb:rb + kr, t * Nc + rt * P: t * Nc + (rt + 1) * P]
                    nc.tensor.matmul(
                        psum[:, c0:c1], lhsT, ws[rb:rb + kr, wc:wc + (c1 - c0)],
                        start=start, stop=stop)
                dst = out_sb[:, seg["out_base"]:seg["out_base"] + L]
                if ev % 2 == 0:
                    nc.scalar.copy(dst, psum[:, :L])
                else:
                    nc.vector.tensor_copy(out=dst, in_=psum[:, :L])
                ev += 1
                # flush evicted output in ~0.5-1MB chunks to overlap the
                # store DMA with remaining compute
                done = seg["out_base"] + L
                if done - flushed >= 2048 or si == len(meta["segments"]) - 1:
                    # ACT's HWDGE ring: independent FIFO from the input
                    # stream on SP, so stores don't head-block behind loads
                    nc.scalar.dma_start(
                        out=out_d[rt * P:(rt + 1) * P, flushed:done],
                        in_=out_sb[:, flushed:done])
                    flushed = done
    nc.compile()
    return nc


def _host_tensors(meta, x2, weight):
    """Build per-core xt and shared ws host arrays (values only)."""
    BS = meta["BS"]
    Nc = meta["rows_per_core"]
    Ntot = Nc * N_CORES

    if x2.shape[0] < Ntot:
        x2 = np.concatenate(
            [x2, np.zeros((Ntot - x2.shape[0], x2.shape[1]), np.float32)], axis=0)

    # ws (shared): [128, ws_cols]
    ws = np.zeros((P, meta["ws_cols"]), np.float32)
    wsum = {}
    for (ob_ib, ks) in meta["wslots"].items():
        w = weight[ks[0]]
        for k in ks[1:]:
            w = w + weight[k]
        wsum[ob_ib] = np.ascontiguousarray(w, dtype=np.float32)
    for (wc, rb, uibs, seg_obs) in meta["ws_blocks"]:
        for r, ib in enumerate(uibs):
            row0 = rb + r * 64
            for j, ob in enumerate(seg_obs):
                w = wsum.get((ob, ib))
                if w is not None:
                    ws[row0:row0 + 64, wc + j * BS: wc + (j + 1) * BS] = w

    # xt per core: [128, n_tiles*Nc]; tile t covers cols [t*Nc, (t+1)*Nc)
    xt_all = []
    for c in range(N_CORES):
        xs = x2[c * Nc:(c + 1) * Nc]           # [Nc, F]
        xt = np.zeros((P, len(meta["xt_tiles"]) * Nc), np.float32)
        for t, entries in enumerate(meta["xt_tiles"]):
            for (rbase, ib) in entries:
                xt[rbase:rbase + 64, t * Nc:(t + 1) * Nc] = \
                    xs[:, ib * BS:(ib + 1) * BS].T
        xt_all.append(np.ascontiguousarray(xt))
    return xt_all, np.ascontiguousarray(ws)


def kernel(**inputs):
    global LAST_RESULT
    x = np.asarray(inputs["x"], dtype=np.float32)
    weight = np.asarray(inputs["weight"], dtype=np.float32)
    bias = np.asarray(inputs["bias"], dtype=np.float32)
    out_idx = np.asarray(inputs["out_block_idx"]).astype(np.int64)
    in_idx = np.asarray(inputs["in_block_idx"]).astype(np.int64)

    B, S, F = x.shape
    N = B * S
    BS = weight.shape[1]
    OUT_F = bias.shape[0]
    x2 = np.ascontiguousarray(x.reshape(N, F))

    key = (N, F, OUT_F, BS, out_idx.tobytes(), in_idx.tobytes())
    if key not in _CACHE:
        meta = _build_schedule(N, F, OUT_F, BS, out_idx, in_idx)
        nc = _build_nc(meta)
        _CACHE[key] = (nc, meta)
    nc, meta = _CACHE[key]

    xt_all, ws = _host_tensors(meta, x2, weight)
    in_maps = [{"xt": xt_all[c], "ws": ws} for c in range(N_CORES)]
    res = bass_utils.run_bass_kernel_spmd(nc, in_maps, core_ids=list(range(N_CORES)))
    LAST_RESULT = res

    Nc = meta["rows_per_core"]
    dev = np.concatenate([res.results[c]["out"] for c in range(N_CORES)], axis=0)
    dev = dev[:N]  # drop row padding

    out = np.zeros((N, OUT_F), np.float32)
    for seg in meta["segments"]:
        b = seg["out_base"]
        for j, ob in enumerate(seg["obs"]):
            out[:, ob * BS:(ob + 1) * BS] = dev[:, b + j * BS: b + (j + 1) * BS]
    if bias.any():
        out += bias
    return out.reshape(B, S, OUT_F)


# revision 16
# speedup vs baseline: 1.4284x; 1.4284x over previous
"""Block-sparse linear kernel for Trainium2 (8 NeuronCores, Bass/Tile).

Computes out[n, ob*BS:(ob+1)*BS] += x[n, ib*BS:(ib+1)*BS] @ W[k]  for each
nonzero block k with indices (ob, ib), plus bias — data-parallel over the
flattened row dim N across 8 cores (weights/indices replicated).

Strategy (host-side schedule specialization from the index tensors):
  - Group input-blocks (ibs) into *families* with identical sets of
    output-blocks (obs).  Families whose obs-sets overlap are merged into
    *superfamilies* (zero-filled weight stacking keeps those correct).
  - Within a family, pair up ibs: a pair forms one K=128 stationary
    operand (the pair's two 64-feature slices of x, transposed host-side),
    streaming a [128, n_obs*64] stacked weight -> full PE utilization.
  - PSUM accumulates each superfamily-segment (<=16 obs = <=1024 f32 cols
    = 2 banks) over all its pairs/singles via matmul start/stop flags.
  - Output is laid out family-major (contiguous evictions); the host
    un-permutes output block columns and adds bias.
  - Matmuls run in float32r (TF32-like, ~1.5e-4 rel err, full PE rate).

The device kernel does: 2 input DMAs, matmul accumulation, PSUM->SBUF
evictions on ACT/DVE, 1 output DMA per 128-row tile.  All x transposition
and index logic happens on the host at schedule-build time.
"""

import os
import numpy as np
import ml_dtypes
from collections import defaultdict
from contextlib import ExitStack

from concourse import bass_utils, bacc, mybir
import concourse.tile as tile

N_CORES = 8
P = 128            # partitions / row-tile size
SEG_MAX_OBS = 16   # psum segment cap: 16 blocks * 64 = 1024 f32 = 2 banks
F32R = mybir.dt.float32r
F32 = mybir.dt.float32
BF16 = mybir.dt.bfloat16

# input dtype for the tensor engine: bf16 (default) halves input DMA and
# gets fast weight loads (~2.8e-3 rel err); f32r is TF32-like (~1.5e-4)
KDTYPE = os.environ.get("KDTYPE", "bf16")
DT_IN = BF16 if KDTYPE == "bf16" else F32R
NP_IN = ml_dtypes.bfloat16 if KDTYPE == "bf16" else np.float32

# schedule-key -> (nc, meta) cache so repeated kernel() calls reuse the
# compiled module (and the NEFF cache underneath).
_CACHE = {}

# test harness introspection: last BassKernelResults
LAST_RESULT = None


def _build_schedule(N, F, OUT_F, BS, out_idx, in_idx):
    """Pure-index schedule: families, pairs, segments, layouts."""
    n_ib = F // BS
    n_ob = OUT_F // BS
    assert F % BS == 0 and OUT_F % BS == 0

    # (ob, ib) -> list of weight slots k (duplicates summed host-side)
    wslots = defaultdict(list)
    for k, (ob, ib) in enumerate(zip(out_idx, in_idx)):
        ob, ib = int(ob), int(ib)
        assert 0 <= ob < n_ob and 0 <= ib < n_ib
        wslots[(ob, ib)].append(k)

    obs_by_ib = defaultdict(set)
    for (ob, ib) in wslots:
        obs_by_ib[ib].add(ob)

    # families: ibs with identical obs sets
    fam_map = defaultdict(list)
    for ib in sorted(obs_by_ib):
        fam_map[frozenset(obs_by_ib[ib])].append(ib)
    families = [(sorted(obs), ibs) for obs, ibs in fam_map.items()]

    # union-find over obs to merge overlapping families into superfamilies
    parent = {}

    def find(a):
        while parent[a] != a:
            parent[a] = parent[parent[a]]
            a = parent[a]
        return a

    for obs, _ in families:
        for ob in obs:
            parent.setdefault(ob, ob)
        r = find(obs[0])
        for ob in obs[1:]:
            parent[find(ob)] = r
    sf_map = defaultdict(lambda: {"obs": set(), "fams": []})
    for obs, ibs in families:
        root = find(obs[0])
        sf_map[root]["obs"].update(obs)
        sf_map[root]["fams"].append((obs, ibs))
    superfams = sorted(sf_map.values(), key=lambda s: min(s["obs"]))

    # xt tile table: pairs (full K=128) and packed singles (K=64 halves)
    xt_tiles = []      # per tile: list of (rowbase, ib) entries
    unit_of = {}       # (fam_id, pair_idx) -> (tile_idx, rowbase, krows, ibs)
    singles = []       # deferred: (fam_key, ib)
    fam_units = defaultdict(list)   # fam key -> [(tile, rowbase, krows, ibs)]
    fam_id = 0
    fam_keys = {}
    for sf in superfams:
        for obs, ibs in sf["fams"]:
            key = fam_id
            fam_keys[key] = (tuple(obs), tuple(ibs))
            for i in range(0, len(ibs) - 1, 2):
                t = len(xt_tiles)
                xt_tiles.append([(0, ibs[i]), (64, ibs[i + 1])])
                fam_units[key].append((t, 0, 128, (ibs[i], ibs[i + 1])))
            if len(ibs) % 2:
                singles.append((key, ibs[-1]))
            fam_id += 1
    for j in range(0, len(singles), 2):
        t = len(xt_tiles)
        entries = [(0, singles[j][1])]
        fam_units[singles[j][0]].append((t, 0, 64, (singles[j][1],)))
        if j + 1 < len(singles):
            entries.append((64, singles[j + 1][1]))
            fam_units[singles[j + 1][0]].append((t, 64, 64, (singles[j + 1][1],)))
        xt_tiles.append(entries)

    # segments + ws layout + out layout
    # mm task: (psum_c0, psum_c1, tile, rowbase, krows, ws_c0, start, stop)
    segments = []   # per segment: dict(out_base, n_obs, obs, tasks)
    ws_blocks = []  # (ws_col, rowbase, ib_or_None, obs_list) for host fill
    ws_cols = 0
    out_cols = 0
    fid = 0
    for sf in superfams:
        sf_obs = sorted(sf["obs"])
        # family units of this superfamily, in deterministic order
        units = []
        base = fid
        for obs, ibs in sf["fams"]:
            units.append((fid, tuple(obs)))
            fid += 1
        for s0 in range(0, len(sf_obs), SEG_MAX_OBS):
            seg_obs = sf_obs[s0:s0 + SEG_MAX_OBS]
            L = len(seg_obs) * BS
            tasks = []
            all_units = []
            for key, fobs in units:
                for (t, rb, kr, uibs) in fam_units[key]:
                    all_units.append((t, rb, kr, uibs))
            seg_ws0 = ws_cols
            unit_ws = []
            unit_tiles = []
            for ui, (t, rb, kr, uibs) in enumerate(all_units):
                wc = ws_cols
                ws_blocks.append((wc, rb, uibs, seg_obs))
                unit_ws.append((wc, wc + L))
                unit_tiles.append(t)
                for c0 in range(0, L, 512):
                    c1 = min(c0 + 512, L)
                    tasks.append((c0, c1, t, rb, kr, wc + c0,
                                  ui == 0, ui == len(all_units) - 1))
                ws_cols += L
            segments.append({"out_base": out_cols, "n_obs": len(seg_obs),
                             "obs": seg_obs, "tasks": tasks,
                             "ws0": seg_ws0, "ws1": ws_cols,
                             "unit_ws": unit_ws, "unit_tiles": unit_tiles,
                             "tiles": sorted({tk[2] for tk in tasks})})
            out_cols += L

    n_pad = (-N) % (N_CORES * P)
    rows_per_core = (N + n_pad) // N_CORES
    rt_count = rows_per_core // P

    # input-DMA load plan in consumption order: ("ws"|"xt", c0, c1).
    # xt entries are tile-index ranges; first segment's ws goes per-unit so
    # the very first matmul only waits on a small chunk.
    load_plan = []
    seen_tiles = set()

    def add_tiles(tiles):
        new_t = [t for t in tiles if t not in seen_tiles]
        seen_tiles.update(new_t)
        i = 0
        while i < len(new_t):   # merge consecutive tile indices into ranges
            j = i
            while j + 1 < len(new_t) and new_t[j + 1] == new_t[j] + 1:
                j += 1
            load_plan.append(("xt", new_t[i], new_t[j] + 1))
            i = j + 1

    for si, seg in enumerate(segments):
        if si == 0:
            # finest interleave: each unit's ws chunk followed by its xt
            # tile, so the first matmul waits on ~0.7MB only
            for (a, b), t in zip(seg["unit_ws"], seg["unit_tiles"]):
                load_plan.append(("ws", a, b))
                add_tiles([t])
        else:
            load_plan.append(("ws", seg["ws0"], seg["ws1"]))
            add_tiles(seg["tiles"])

    return {
        "N": N, "F": F, "OUT_F": OUT_F, "BS": BS,
        "wslots": dict(wslots),
        "xt_tiles": xt_tiles,
        "ws_blocks": ws_blocks, "ws_cols": ws_cols,
        "segments": segments, "out_cols": out_cols,
        "rows_per_core": rows_per_core, "rt_count": rt_count,
        "load_plan": load_plan,
    }


def _build_nc(meta):
    """Emit the Bass/Tile module for a schedule (value-independent)."""
    Nc = meta["rows_per_core"]
    XTC = len(meta["xt_tiles"]) * Nc
    WSC = meta["ws_cols"]
    OUTC = meta["out_cols"]
    rt_count = meta["rt_count"]

    nc = bacc.Bacc("TRN2", target_bir_lowering=False, debug=False)
    xt_d = nc.dram_tensor("xt", [P, XTC], DT_IN, kind="ExternalInput")
    ws_d = nc.dram_tensor("ws", [P, WSC], DT_IN, kind="ExternalInput")
    out_d = nc.dram_tensor("out", [Nc, OUTC], F32, kind="ExternalOutput")

    n_warm = int(os.environ.get("KWARM", "8"))
    flush_cols = int(os.environ.get("KFLUSH", "1600"))

    with tile.TileContext(nc) as tc, ExitStack() as ctx:
        xt_pool = ctx.enter_context(tc.tile_pool(name="xt", bufs=1))
        ws_pool = ctx.enter_context(tc.tile_pool(name="ws", bufs=1))
        warm_pool = ctx.enter_context(tc.tile_pool(name="wm", bufs=1))
        psum_pool = ctx.enter_context(tc.tile_pool(name="ps", bufs=4, space="PSUM"))
        out_pool = ctx.enter_context(tc.tile_pool(name="ot", bufs=1))

        xt = xt_pool.tile([P, XTC], DT_IN)
        ws = ws_pool.tile([P, WSC], DT_IN)

        # PE warm-up: dummy matmuls with no DMA deps run during the input
        # load head and flip HAM to 8/8 before the first real matmul.
        if n_warm:
            wsb = warm_pool.tile([P, 512], DT_IN)
            nc.vector.memset(wsb[:].bitcast(F32), 0)
            wps = psum_pool.tile([P, 1024], F32, tag="mm")
            for _ in range(n_warm):
                nc.tensor.matmul(wps[:, :512], wsb[:, :P], wsb[:, :512],
                                 start=True, stop=True)

        # chunked input DMAs in first-use order so matmuls start early
        for (kind, a, b) in meta["load_plan"]:
            if kind == "ws":
                nc.sync.dma_start(out=ws[:, a:b], in_=ws_d[:, a:b])
            else:
                nc.sync.dma_start(out=xt[:, a * Nc:b * Nc], in_=xt_d[:, a * Nc:b * Nc])

        # segment-outer / row-tile-inner: each segment's data is consumed
        # for all row tiles right after it lands, so the PE runs dense and
        # stays ahead of the input stream instead of idling through it.
        out_sbs = [out_pool.tile([P, OUTC], F32, name=f"osb{r}", tag=f"osb{r}")
                   for r in range(rt_count)]
        flushed = [0] * rt_count
        ev = 0
        for si, seg in enumerate(meta["segments"]):
            L = seg["n_obs"] * meta["BS"]
            last = si == len(meta["segments"]) - 1
            for rt in range(rt_count):
                psum = psum_pool.tile([P, 1024], F32, tag="mm")
                for (c0, c1, t, rb, kr, wc, start, stop) in seg["tasks"]:
                    lhsT = xt[rb:rb + kr, t * Nc + rt * P: t * Nc + (rt + 1) * P]
                    nc.tensor.matmul(
                        psum[:, c0:c1], lhsT, ws[rb:rb + kr, wc:wc + (c1 - c0)],
                        start=start, stop=stop)
                dst = out_sbs[rt][:, seg["out_base"]:seg["out_base"] + L]
                if ev % 2 == 0:
                    nc.scalar.copy(dst, psum[:, :L])
                else:
                    nc.vector.tensor_copy(out=dst, in_=psum[:, :L])
                ev += 1
                # flush finished output columns in ~0.8MB chunks; the store
                # stream shares SP's HWDGE FIFO, so it naturally yields to
                # the (critical) input stream queued ahead of it
                done = seg["out_base"] + L
                if done - flushed[rt] >= flush_cols or last:
                    nc.sync.dma_start(
                        out=out_d[rt * P:(rt + 1) * P, flushed[rt]:done],
                        in_=out_sbs[rt][:, flushed[rt]:done])
                    flushed[rt] = done
    nc.compile()
    return nc


def _host_tensors(meta, x2, weight):
    """Build per-core xt and shared ws host arrays (values only)."""
    BS = meta["BS"]
    Nc = meta["rows_per_core"]
    Ntot = Nc * N_CORES

    if x2.shape[0] < Ntot:
        x2 = np.concatenate(
            [x2, np.zeros((Ntot - x2.shape[0], x2.shape[1]), np.float32)], axis=0)

    # ws (shared): [128, ws_cols]
    ws = np.zeros((P, meta["ws_cols"]), np.float32)
    wsum = {}
    for (ob_ib, ks) in meta["wslots"].items():
        w = weight[ks[0]]
        for k in ks[1:]:
            w = w + weight[k]
        wsum[ob_ib] = np.ascontiguousarray(w, dtype=np.float32)
    for (wc, rb, uibs, seg_obs) in meta["ws_blocks"]:
        for r, ib in enumerate(uibs):
            row0 = rb + r * 64
            for j, ob in enumerate(seg_obs):
                w = wsum.get((ob, ib))
                if w is not None:
                    ws[row0:row0 + 64, wc + j * BS: wc + (j + 1) * BS] = w

    # xt per core: [128, n_tiles*Nc]; tile t covers cols [t*Nc, (t+1)*Nc)
    xt_all = []
    for c in range(N_CORES):
        xs = x2[c * Nc:(c + 1) * Nc]           # [Nc, F]
        xt = np.zeros((P, len(meta["xt_tiles"]) * Nc), np.float32)
        for t, entries in enumerate(meta["xt_tiles"]):
            for (rbase, ib) in entries:
                xt[rbase:rbase + 64, t * Nc:(t + 1) * Nc] = \
                    xs[:, ib * BS:(ib + 1) * BS].T
        xt_all.append(np.ascontiguousarray(xt.astype(NP_IN)))
    return xt_all, np.ascontiguousarray(ws.astype(NP_IN))


def kernel(**inputs):
    global LAST_RESULT
    x = np.asarray(inputs["x"], dtype=np.float32)
    weight = np.asarray(inputs["weight"], dtype=np.float32)
    bias = np.asarray(inputs["bias"], dtype=np.float32)
    out_idx = np.asarray(inputs["out_block_idx"]).astype(np.int64)
    in_idx = np.asarray(inputs["in_block_idx"]).astype(np.int64)

    B, S, F = x.shape
    N = B * S
    BS = weight.shape[1]
    OUT_F = bias.shape[0]
    x2 = np.ascontiguousarray(x.reshape(N, F))

    key = (N, F, OUT_F, BS, out_idx.tobytes(), in_idx.tobytes())
    if key not in _CACHE:
        meta = _build_schedule(N, F, OUT_F, BS, out_idx, in_idx)
        nc = _build_nc(meta)
        _CACHE[key] = (nc, meta)
    nc, meta = _CACHE[key]

    xt_all, ws = _host_tensors(meta, x2, weight)
    in_maps = [{"xt": xt_all[c], "ws": ws} for c in range(N_CORES)]
    res = bass_utils.run_bass_kernel_spmd(nc, in_maps, core_ids=list(range(N_CORES)))
    LAST_RESULT = res

    Nc = meta["rows_per_core"]
    dev = np.concatenate([res.results[c]["out"] for c in range(N_CORES)], axis=0)
    dev = dev[:N]  # drop row padding

    out = np.zeros((N, OUT_F), np.float32)
    for seg in meta["segments"]:
        b = seg["out_base"]
        for j, ob in enumerate(seg["obs"]):
            out[:, ob * BS:(ob + 1) * BS] = dev[:, b + j * BS: b + (j + 1) * BS]
    if bias.any():
        out += bias
    return out.reshape(B, S, OUT_F)


# revision 17
# speedup vs baseline: 1.7243x; 1.2071x over previous
"""Block-sparse linear kernel for Trainium2 (8 NeuronCores, Bass/Tile).

Computes out[n, ob*BS:(ob+1)*BS] += x[n, ib*BS:(ib+1)*BS] @ W[k]  for each
nonzero block k with indices (ob, ib), plus bias — data-parallel over the
flattened row dim N across 8 cores (weights/indices replicated).

Strategy (host-side schedule specialization from the index tensors):
  - Group input-blocks (ibs) into *families* with identical sets of
    output-blocks (obs).  Families whose obs-sets overlap are merged into
    *superfamilies* (zero-filled weight stacking keeps those correct).
  - Within a family, pair up ibs: a pair forms one K=128 stationary
    operand (the pair's two 64-feature slices of x, transposed host-side),
    streaming a [128, n_obs*64] stacked weight -> full PE utilization.
  - PSUM accumulates each superfamily-segment (<=16 obs = <=1024 f32 cols
    = 2 banks) over all its pairs/singles via matmul start/stop flags.
  - Output is laid out family-major (contiguous evictions); the host
    un-permutes output block columns and adds bias.
  - Matmuls run in float32r (TF32-like, ~1.5e-4 rel err, full PE rate).

The device kernel does: 2 input DMAs, matmul accumulation, PSUM->SBUF
evictions on ACT/DVE, 1 output DMA per 128-row tile.  All x transposition
and index logic happens on the host at schedule-build time.
"""

import os
import numpy as np
import ml_dtypes
from collections import defaultdict
from contextlib import ExitStack

from concourse import bass_utils, bacc, mybir
import concourse.tile as tile

N_CORES = 8
P = 128            # partitions / row-tile size
SEG_MAX_OBS = 16   # psum segment cap: 16 blocks * 64 = 1024 f32 = 2 banks
F32R = mybir.dt.float32r
F32 = mybir.dt.float32
BF16 = mybir.dt.bfloat16

# input dtype for the tensor engine: bf16 (default) halves input DMA and
# gets fast weight loads (~2.8e-3 rel err); f32r is TF32-like (~1.5e-4)
KDTYPE = os.environ.get("KDTYPE", "bf16")
DT_IN = BF16 if KDTYPE == "bf16" else F32R
NP_IN = ml_dtypes.bfloat16 if KDTYPE == "bf16" else np.float32
# output dtype: bf16 halves store traffic (adds ~2e-3 rounding, still far
# under the rel-err gate); f32 is exact
KOUT = os.environ.get("KOUT", "bf16")
DT_OUT = BF16 if KOUT == "bf16" else F32
NP_OUT = ml_dtypes.bfloat16 if KOUT == "bf16" else np.float32

# schedule-key -> (nc, meta) cache so repeated kernel() calls reuse the
# compiled module (and the NEFF cache underneath).
_CACHE = {}

# test harness introspection: last BassKernelResults
LAST_RESULT = None


def _build_schedule(N, F, OUT_F, BS, out_idx, in_idx):
    """Pure-index schedule: families, pairs, segments, layouts."""
    n_ib = F // BS
    n_ob = OUT_F // BS
    assert F % BS == 0 and OUT_F % BS == 0

    # (ob, ib) -> list of weight slots k (duplicates summed host-side)
    wslots = defaultdict(list)
    for k, (ob, ib) in enumerate(zip(out_idx, in_idx)):
        ob, ib = int(ob), int(ib)
        assert 0 <= ob < n_ob and 0 <= ib < n_ib
        wslots[(ob, ib)].append(k)

    obs_by_ib = defaultdict(set)
    for (ob, ib) in wslots:
        obs_by_ib[ib].add(ob)

    # families: ibs with identical obs sets
    fam_map = defaultdict(list)
    for ib in sorted(obs_by_ib):
        fam_map[frozenset(obs_by_ib[ib])].append(ib)
    families = [(sorted(obs), ibs) for obs, ibs in fam_map.items()]

    # union-find over obs to merge overlapping families into superfamilies
    parent = {}

    def find(a):
        while parent[a] != a:
            parent[a] = parent[parent[a]]
            a = parent[a]
        return a

    for obs, _ in families:
        for ob in obs:
            parent.setdefault(ob, ob)
        r = find(obs[0])
        for ob in obs[1:]:
            parent[find(ob)] = r
    sf_map = defaultdict(lambda: {"obs": set(), "fams": []})
    for obs, ibs in families:
        root = find(obs[0])
        sf_map[root]["obs"].update(obs)
        sf_map[root]["fams"].append((obs, ibs))
    superfams = sorted(sf_map.values(), key=lambda s: min(s["obs"]))

    # xt tile table: pairs (full K=128) and packed singles (K=64 halves)
    xt_tiles = []      # per tile: list of (rowbase, ib) entries
    unit_of = {}       # (fam_id, pair_idx) -> (tile_idx, rowbase, krows, ibs)
    singles = []       # deferred: (fam_key, ib)
    fam_units = defaultdict(list)   # fam key -> [(tile, rowbase, krows, ibs)]
    fam_id = 0
    fam_keys = {}
    for sf in superfams:
        for obs, ibs in sf["fams"]:
            key = fam_id
            fam_keys[key] = (tuple(obs), tuple(ibs))
            for i in range(0, len(ibs) - 1, 2):
                t = len(xt_tiles)
                xt_tiles.append([(0, ibs[i]), (64, ibs[i + 1])])
                fam_units[key].append((t, 0, 128, (ibs[i], ibs[i + 1])))
            if len(ibs) % 2:
                singles.append((key, ibs[-1]))
            fam_id += 1
    for j in range(0, len(singles), 2):
        t = len(xt_tiles)
        entries = [(0, singles[j][1])]
        fam_units[singles[j][0]].append((t, 0, 64, (singles[j][1],)))
        if j + 1 < len(singles):
            entries.append((64, singles[j + 1][1]))
            fam_units[singles[j + 1][0]].append((t, 64, 64, (singles[j + 1][1],)))
        xt_tiles.append(entries)

    # segments + ws layout + out layout
    # mm task: (psum_c0, psum_c1, tile, rowbase, krows, ws_c0, start, stop)
    segments = []   # per segment: dict(out_base, n_obs, obs, tasks)
    ws_blocks = []  # (ws_col, rowbase, ib_or_None, obs_list) for host fill
    ws_cols = 0
    out_cols = 0
    fid = 0
    for sf in superfams:
        sf_obs = sorted(sf["obs"])
        # family units of this superfamily, in deterministic order
        units = []
        base = fid
        for obs, ibs in sf["fams"]:
            units.append((fid, tuple(obs)))
            fid += 1
        for s0 in range(0, len(sf_obs), SEG_MAX_OBS):
            seg_obs = sf_obs[s0:s0 + SEG_MAX_OBS]
            L = len(seg_obs) * BS
            tasks = []
            all_units = []
            for key, fobs in units:
                for (t, rb, kr, uibs) in fam_units[key]:
                    all_units.append((t, rb, kr, uibs))
            seg_ws0 = ws_cols
            unit_ws = []
            unit_tiles = []
            for ui, (t, rb, kr, uibs) in enumerate(all_units):
                wc = ws_cols
                ws_blocks.append((wc, rb, uibs, seg_obs))
                unit_ws.append((wc, wc + L))
                unit_tiles.append(t)
                for c0 in range(0, L, 512):
                    c1 = min(c0 + 512, L)
                    tasks.append((c0, c1, t, rb, kr, wc + c0,
                                  ui == 0, ui == len(all_units) - 1))
                ws_cols += L
            segments.append({"out_base": out_cols, "n_obs": len(seg_obs),
                             "obs": seg_obs, "tasks": tasks,
                             "ws0": seg_ws0, "ws1": ws_cols,
                             "unit_ws": unit_ws, "unit_tiles": unit_tiles,
                             "tiles": sorted({tk[2] for tk in tasks})})
            out_cols += L

    n_pad = (-N) % (N_CORES * P)
    rows_per_core = (N + n_pad) // N_CORES
    rt_count = rows_per_core // P

    # input-DMA load plan in consumption order: ("ws"|"xt", c0, c1).
    # xt entries are tile-index ranges; first segment's ws goes per-unit so
    # the very first matmul only waits on a small chunk.
    load_plan = []
    seen_tiles = set()

    def add_tiles(tiles):
        new_t = [t for t in tiles if t not in seen_tiles]
        seen_tiles.update(new_t)
        i = 0
        while i < len(new_t):   # merge consecutive tile indices into ranges
            j = i
            while j + 1 < len(new_t) and new_t[j + 1] == new_t[j] + 1:
                j += 1
            load_plan.append(("xt", new_t[i], new_t[j] + 1))
            i = j + 1

    for si, seg in enumerate(segments):
        if si == 0:
            # finest interleave: each unit's ws chunk followed by its xt
            # tile, so the first matmul waits on ~0.7MB only
            for (a, b), t in zip(seg["unit_ws"], seg["unit_tiles"]):
                load_plan.append(("ws", a, b))
                add_tiles([t])
        else:
            load_plan.append(("ws", seg["ws0"], seg["ws1"]))
            add_tiles(seg["tiles"])

    return {
        "N": N, "F": F, "OUT_F": OUT_F, "BS": BS,
        "wslots": dict(wslots),
        "xt_tiles": xt_tiles,
        "ws_blocks": ws_blocks, "ws_cols": ws_cols,
        "segments": segments, "out_cols": out_cols,
        "rows_per_core": rows_per_core, "rt_count": rt_count,
        "load_plan": load_plan,
    }


def _build_nc(meta):
    """Emit the Bass/Tile module for a schedule (value-independent)."""
    Nc = meta["rows_per_core"]
    XTC = len(meta["xt_tiles"]) * Nc
    WSC = meta["ws_cols"]
    OUTC = meta["out_cols"]
    rt_count = meta["rt_count"]

    nc = bacc.Bacc("TRN2", target_bir_lowering=False, debug=False)
    xt_d = nc.dram_tensor("xt", [P, XTC], DT_IN, kind="ExternalInput")
    ws_d = nc.dram_tensor("ws", [P, WSC], DT_IN, kind="ExternalInput")
    out_d = nc.dram_tensor("out", [Nc, OUTC], DT_OUT, kind="ExternalOutput")

    n_warm = int(os.environ.get("KWARM", "8"))
    flush_cols = int(os.environ.get("KFLUSH", "800"))

    with tile.TileContext(nc) as tc, ExitStack() as ctx:
        xt_pool = ctx.enter_context(tc.tile_pool(name="xt", bufs=1))
        ws_pool = ctx.enter_context(tc.tile_pool(name="ws", bufs=1))
        warm_pool = ctx.enter_context(tc.tile_pool(name="wm", bufs=1))
        psum_pool = ctx.enter_context(tc.tile_pool(name="ps", bufs=4, space="PSUM"))
        out_pool = ctx.enter_context(tc.tile_pool(name="ot", bufs=1))

        xt = xt_pool.tile([P, XTC], DT_IN)
        ws = ws_pool.tile([P, WSC], DT_IN)

        # PE warm-up: dummy matmuls with no DMA deps run during the input
        # load head and flip HAM to 8/8 before the first real matmul.
        if n_warm:
            wsb = warm_pool.tile([P, 512], DT_IN)
            nc.gpsimd.memset(wsb[:].bitcast(F32), 0)
            wps = psum_pool.tile([P, 1024], F32, tag="mm")
            for _ in range(n_warm):
                nc.tensor.matmul(wps[:, :512], wsb[:, :P], wsb[:, :512],
                                 start=True, stop=True)

        # chunked input DMAs in first-use order so matmuls start early
        for (kind, a, b) in meta["load_plan"]:
            if kind == "ws":
                nc.sync.dma_start(out=ws[:, a:b], in_=ws_d[:, a:b])
            else:
                nc.sync.dma_start(out=xt[:, a * Nc:b * Nc], in_=xt_d[:, a * Nc:b * Nc])

        # segment-outer / row-tile-inner: each segment's data is consumed
        # for all row tiles right after it lands, so the PE runs dense and
        # stays ahead of the input stream instead of idling through it.
        out_sbs = [out_pool.tile([P, OUTC], DT_OUT, name=f"osb{r}", tag=f"osb{r}")
                   for r in range(rt_count)]
        flushed = [0] * rt_count
        ev = 0
        for si, seg in enumerate(meta["segments"]):
            L = seg["n_obs"] * meta["BS"]
            last = si == len(meta["segments"]) - 1
            for rt in range(rt_count):
                psum = psum_pool.tile([P, 1024], F32, tag="mm")
                for (c0, c1, t, rb, kr, wc, start, stop) in seg["tasks"]:
                    lhsT = xt[rb:rb + kr, t * Nc + rt * P: t * Nc + (rt + 1) * P]
                    nc.tensor.matmul(
                        psum[:, c0:c1], lhsT, ws[rb:rb + kr, wc:wc + (c1 - c0)],
                        start=start, stop=stop)
                dst = out_sbs[rt][:, seg["out_base"]:seg["out_base"] + L]
                if ev % 2 == 0:
                    nc.scalar.copy(dst, psum[:, :L])
                else:
                    nc.vector.tensor_copy(out=dst, in_=psum[:, :L])
                ev += 1
                # flush finished output columns in ~0.8MB chunks; the store
                # stream shares SP's HWDGE FIFO, so it naturally yields to
                # the (critical) input stream queued ahead of it
                done = seg["out_base"] + L
                if done - flushed[rt] >= flush_cols or last:
                    nc.sync.dma_start(
                        out=out_d[rt * P:(rt + 1) * P, flushed[rt]:done],
                        in_=out_sbs[rt][:, flushed[rt]:done])
                    flushed[rt] = done
    nc.compile()
    return nc


def _host_tensors(meta, x2, weight):
    """Build per-core xt and shared ws host arrays (values only)."""
    BS = meta["BS"]
    Nc = meta["rows_per_core"]
    Ntot = Nc * N_CORES

    if x2.shape[0] < Ntot:
        x2 = np.concatenate(
            [x2, np.zeros((Ntot - x2.shape[0], x2.shape[1]), np.float32)], axis=0)

    # ws (shared): [128, ws_cols]
    ws = np.zeros((P, meta["ws_cols"]), np.float32)
    wsum = {}
    for (ob_ib, ks) in meta["wslots"].items():
        w = weight[ks[0]]
        for k in ks[1:]:
            w = w + weight[k]
        wsum[ob_ib] = np.ascontiguousarray(w, dtype=np.float32)
    for (wc, rb, uibs, seg_obs) in meta["ws_blocks"]:
        for r, ib in enumerate(uibs):
            row0 = rb + r * 64
            for j, ob in enumerate(seg_obs):
                w = wsum.get((ob, ib))
                if w is not None:
                    ws[row0:row0 + 64, wc + j * BS: wc + (j + 1) * BS] = w

    # xt per core: [128, n_tiles*Nc]; tile t covers cols [t*Nc, (t+1)*Nc)
    xt_all = []
    for c in range(N_CORES):
        xs = x2[c * Nc:(c + 1) * Nc]           # [Nc, F]
        xt = np.zeros((P, len(meta["xt_tiles"]) * Nc), np.float32)
        for t, entries in enumerate(meta["xt_tiles"]):
            for (rbase, ib) in entries:
                xt[rbase:rbase + 64, t * Nc:(t + 1) * Nc] = \
                    xs[:, ib * BS:(ib + 1) * BS].T
        xt_all.append(np.ascontiguousarray(xt.astype(NP_IN)))
    return xt_all, np.ascontiguousarray(ws.astype(NP_IN))


def kernel(**inputs):
    global LAST_RESULT
    x = np.asarray(inputs["x"], dtype=np.float32)
    weight = np.asarray(inputs["weight"], dtype=np.float32)
    bias = np.asarray(inputs["bias"], dtype=np.float32)
    out_idx = np.asarray(inputs["out_block_idx"]).astype(np.int64)
    in_idx = np.asarray(inputs["in_block_idx"]).astype(np.int64)

    B, S, F = x.shape
    N = B * S
    BS = weight.shape[1]
    OUT_F = bias.shape[0]
    x2 = np.ascontiguousarray(x.reshape(N, F))

    key = (N, F, OUT_F, BS, out_idx.tobytes(), in_idx.tobytes())
    if key not in _CACHE:
        meta = _build_schedule(N, F, OUT_F, BS, out_idx, in_idx)
        nc = _build_nc(meta)
        _CACHE[key] = (nc, meta)
    nc, meta = _CACHE[key]

    xt_all, ws = _host_tensors(meta, x2, weight)
    in_maps = [{"xt": xt_all[c], "ws": ws} for c in range(N_CORES)]
    res = bass_utils.run_bass_kernel_spmd(nc, in_maps, core_ids=list(range(N_CORES)))
    LAST_RESULT = res

    Nc = meta["rows_per_core"]
    dev = np.concatenate(
        [np.asarray(res.results[c]["out"]).astype(np.float32)
         for c in range(N_CORES)], axis=0)
    dev = dev[:N]  # drop row padding

    out = np.zeros((N, OUT_F), np.float32)
    for seg in meta["segments"]:
        b = seg["out_base"]
        for j, ob in enumerate(seg["obs"]):
            out[:, ob * BS:(ob + 1) * BS] = dev[:, b + j * BS: b + (j + 1) * BS]
    if bias.any():
        out += bias
    return out.reshape(B, S, OUT_F)


# revision 18
# speedup vs baseline: 1.7435x; 1.0112x over previous
"""Block-sparse linear kernel for Trainium2 (8 NeuronCores, Bass/Tile).

Computes out[n, ob*BS:(ob+1)*BS] += x[n, ib*BS:(ib+1)*BS] @ W[k]  for each
nonzero block k with indices (ob, ib), plus bias — data-parallel over the
flattened row dim N across 8 cores (weights/indices replicated).

Strategy (host-side schedule specialization from the index tensors):
  - Group input-blocks (ibs) into *families* with identical sets of
    output-blocks (obs).  Families whose obs-sets overlap are merged into
    *superfamilies* (zero-filled weight stacking keeps those correct).
  - Within a family, pair up ibs: a pair forms one K=128 stationary
    operand (the pair's two 64-feature slices of x, transposed host-side),
    streaming a [128, n_obs*64] stacked weight -> full PE utilization.
  - PSUM accumulates each superfamily-segment (<=16 obs = <=1024 f32 cols
    = 2 banks) over all its pairs/singles via matmul start/stop flags.
  - Output is laid out family-major (contiguous evictions); the host
    un-permutes output block columns and adds bias.
  - Matmuls run in float32r (TF32-like, ~1.5e-4 rel err, full PE rate).

The device kernel does: 2 input DMAs, matmul accumulation, PSUM->SBUF
evictions on ACT/DVE, 1 output DMA per 128-row tile.  All x transposition
and index logic happens on the host at schedule-build time.
"""

import os
import numpy as np
import ml_dtypes
from collections import defaultdict
from contextlib import ExitStack

from concourse import bass_utils, bacc, mybir
import concourse.tile as tile

N_CORES = 8
P = 128            # partitions / row-tile size
SEG_MAX_OBS = 16   # psum segment cap: 16 blocks * 64 = 1024 f32 = 2 banks
F32R = mybir.dt.float32r
F32 = mybir.dt.float32
BF16 = mybir.dt.bfloat16

# input dtype for the tensor engine: bf16 (default) halves input DMA and
# gets fast weight loads (~2.8e-3 rel err); f32r is TF32-like (~1.5e-4)
KDTYPE = os.environ.get("KDTYPE", "bf16")
DT_IN = BF16 if KDTYPE == "bf16" else F32R
NP_IN = ml_dtypes.bfloat16 if KDTYPE == "bf16" else np.float32
# output dtype: bf16 halves store traffic (adds ~2e-3 rounding, still far
# under the rel-err gate); f32 is exact
KOUT = os.environ.get("KOUT", "bf16")
DT_OUT = BF16 if KOUT == "bf16" else F32
NP_OUT = ml_dtypes.bfloat16 if KOUT == "bf16" else np.float32

# schedule-key -> (nc, meta) cache so repeated kernel() calls reuse the
# compiled module (and the NEFF cache underneath).
_CACHE = {}

# test harness introspection: last BassKernelResults
LAST_RESULT = None


def _build_schedule(N, F, OUT_F, BS, out_idx, in_idx):
    """Pure-index schedule: families, pairs, segments, layouts."""
    n_ib = F // BS
    n_ob = OUT_F // BS
    assert F % BS == 0 and OUT_F % BS == 0

    # (ob, ib) -> list of weight slots k (duplicates summed host-side)
    wslots = defaultdict(list)
    for k, (ob, ib) in enumerate(zip(out_idx, in_idx)):
        ob, ib = int(ob), int(ib)
        assert 0 <= ob < n_ob and 0 <= ib < n_ib
        wslots[(ob, ib)].append(k)

    obs_by_ib = defaultdict(set)
    for (ob, ib) in wslots:
        obs_by_ib[ib].add(ob)

    # families: ibs with identical obs sets
    fam_map = defaultdict(list)
    for ib in sorted(obs_by_ib):
        fam_map[frozenset(obs_by_ib[ib])].append(ib)
    families = [(sorted(obs), ibs) for obs, ibs in fam_map.items()]

    # union-find over obs to merge overlapping families into superfamilies
    parent = {}

    def find(a):
        while parent[a] != a:
            parent[a] = parent[parent[a]]
            a = parent[a]
        return a

    for obs, _ in families:
        for ob in obs:
            parent.setdefault(ob, ob)
        r = find(obs[0])
        for ob in obs[1:]:
            parent[find(ob)] = r
    sf_map = defaultdict(lambda: {"obs": set(), "fams": []})
    for obs, ibs in families:
        root = find(obs[0])
        sf_map[root]["obs"].update(obs)
        sf_map[root]["fams"].append((obs, ibs))
    superfams = sorted(sf_map.values(), key=lambda s: min(s["obs"]))

    # xt tile table: pairs (full K=128) and packed singles (K=64 halves)
    xt_tiles = []      # per tile: list of (rowbase, ib) entries
    unit_of = {}       # (fam_id, pair_idx) -> (tile_idx, rowbase, krows, ibs)
    singles = []       # deferred: (fam_key, ib)
    fam_units = defaultdict(list)   # fam key -> [(tile, rowbase, krows, ibs)]
    fam_id = 0
    fam_keys = {}
    for sf in superfams:
        for obs, ibs in sf["fams"]:
            key = fam_id
            fam_keys[key] = (tuple(obs), tuple(ibs))
            for i in range(0, len(ibs) - 1, 2):
                t = len(xt_tiles)
                xt_tiles.append([(0, ibs[i]), (64, ibs[i + 1])])
                fam_units[key].append((t, 0, 128, (ibs[i], ibs[i + 1])))
            if len(ibs) % 2:
                singles.append((key, ibs[-1]))
            fam_id += 1
    for j in range(0, len(singles), 2):
        t = len(xt_tiles)
        entries = [(0, singles[j][1])]
        fam_units[singles[j][0]].append((t, 0, 64, (singles[j][1],)))
        if j + 1 < len(singles):
            entries.append((64, singles[j + 1][1]))
            fam_units[singles[j + 1][0]].append((t, 64, 64, (singles[j + 1][1],)))
        xt_tiles.append(entries)

    # segments + ws layout + out layout
    # mm task: (psum_c0, psum_c1, tile, rowbase, krows, ws_c0, start, stop)
    segments = []   # per segment: dict(out_base, n_obs, obs, tasks)
    ws_blocks = []  # (ws_col, rowbase, ib_or_None, obs_list) for host fill
    ws_cols = 0
    out_cols = 0
    fid = 0
    for sf in superfams:
        sf_obs = sorted(sf["obs"])
        # family units of this superfamily, in deterministic order
        units = []
        base = fid
        for obs, ibs in sf["fams"]:
            units.append((fid, tuple(obs)))
            fid += 1
        for s0 in range(0, len(sf_obs), SEG_MAX_OBS):
            seg_obs = sf_obs[s0:s0 + SEG_MAX_OBS]
            L = len(seg_obs) * BS
            tasks = []
            all_units = []
            for key, fobs in units:
                for (t, rb, kr, uibs) in fam_units[key]:
                    all_units.append((t, rb, kr, uibs))
            seg_ws0 = ws_cols
            unit_ws = []
            unit_tiles = []
            for ui, (t, rb, kr, uibs) in enumerate(all_units):
                wc = ws_cols
                ws_blocks.append((wc, rb, uibs, seg_obs))
                unit_ws.append((wc, wc + L))
                unit_tiles.append(t)
                for c0 in range(0, L, 512):
                    c1 = min(c0 + 512, L)
                    tasks.append((c0, c1, t, rb, kr, wc + c0,
                                  ui == 0, ui == len(all_units) - 1))
                ws_cols += L
            segments.append({"out_base": out_cols, "n_obs": len(seg_obs),
                             "obs": seg_obs, "tasks": tasks,
                             "ws0": seg_ws0, "ws1": ws_cols,
                             "unit_ws": unit_ws, "unit_tiles": unit_tiles,
                             "tiles": sorted({tk[2] for tk in tasks})})
            out_cols += L

    n_pad = (-N) % (N_CORES * P)
    rows_per_core = (N + n_pad) // N_CORES
    rt_count = rows_per_core // P

    # input-DMA load plan in consumption order: ("ws"|"xt", c0, c1).
    # xt entries are tile-index ranges; first segment's ws goes per-unit so
    # the very first matmul only waits on a small chunk.
    load_plan = []
    seen_tiles = set()

    def add_tiles(tiles):
        new_t = [t for t in tiles if t not in seen_tiles]
        seen_tiles.update(new_t)
        i = 0
        while i < len(new_t):   # merge consecutive tile indices into ranges
            j = i
            while j + 1 < len(new_t) and new_t[j + 1] == new_t[j] + 1:
                j += 1
            load_plan.append(("xt", new_t[i], new_t[j] + 1))
            i = j + 1

    for si, seg in enumerate(segments):
        if si == 0:
            # finest interleave: each unit's ws chunk followed by its xt
            # tile, so the first matmul waits on ~0.7MB only
            for (a, b), t in zip(seg["unit_ws"], seg["unit_tiles"]):
                load_plan.append(("ws", a, b))
                add_tiles([t])
        else:
            load_plan.append(("ws", seg["ws0"], seg["ws1"]))
            add_tiles(seg["tiles"])

    return {
        "N": N, "F": F, "OUT_F": OUT_F, "BS": BS,
        "wslots": dict(wslots),
        "xt_tiles": xt_tiles,
        "ws_blocks": ws_blocks, "ws_cols": ws_cols,
        "segments": segments, "out_cols": out_cols,
        "rows_per_core": rows_per_core, "rt_count": rt_count,
        "load_plan": load_plan,
    }


def _build_nc(meta):
    """Emit the Bass/Tile module for a schedule (value-independent)."""
    Nc = meta["rows_per_core"]
    XTC = len(meta["xt_tiles"]) * Nc
    WSC = meta["ws_cols"]
    OUTC = meta["out_cols"]
    rt_count = meta["rt_count"]

    nc = bacc.Bacc("TRN2", target_bir_lowering=False, debug=False)
    xt_d = nc.dram_tensor("xt", [P, XTC], DT_IN, kind="ExternalInput")
    ws_d = nc.dram_tensor("ws", [P, WSC], DT_IN, kind="ExternalInput")
    out_d = nc.dram_tensor("out", [Nc, OUTC], DT_OUT, kind="ExternalOutput")

    n_warm = int(os.environ.get("KWARM", "8"))
    flush_cols = int(os.environ.get("KFLUSH", "800"))

    with tile.TileContext(nc) as tc, ExitStack() as ctx:
        xt_pool = ctx.enter_context(tc.tile_pool(name="xt", bufs=1))
        ws_pool = ctx.enter_context(tc.tile_pool(name="ws", bufs=1))
        psum_pool = ctx.enter_context(tc.tile_pool(name="ps", bufs=4, space="PSUM"))
        out_pool = ctx.enter_context(tc.tile_pool(name="ot", bufs=1))

        xt = xt_pool.tile([P, XTC], DT_IN)
        ws = ws_pool.tile([P, WSC], DT_IN)

        # chunked input DMAs in first-use order so matmuls start early.
        # After the first ws chunk is issued, emit PE warm-up matmuls on it:
        # they only depend on that one DMA, run while the rest of the input
        # streams in, and flip HAM to 8/8 before the first real matmul.
        for li, (kind, a, b) in enumerate(meta["load_plan"]):
            if kind == "ws":
                nc.sync.dma_start(out=ws[:, a:b], in_=ws_d[:, a:b])
            else:
                nc.sync.dma_start(out=xt[:, a * Nc:b * Nc], in_=xt_d[:, a * Nc:b * Nc])
            if li == 0 and n_warm:
                wcols = min(512, b - a)
                wps = psum_pool.tile([P, 1024], F32, tag="mm")
                for _ in range(n_warm):
                    nc.tensor.matmul(wps[:, :wcols], ws[:, a:a + P],
                                     ws[:, a:a + wcols], start=True, stop=True)

        # segment-outer / row-tile-inner: each segment's data is consumed
        # for all row tiles right after it lands, so the PE runs dense and
        # stays ahead of the input stream instead of idling through it.
        out_sbs = [out_pool.tile([P, OUTC], DT_OUT, name=f"osb{r}", tag=f"osb{r}")
                   for r in range(rt_count)]
        flushed = [0] * rt_count
        ev = 0
        for si, seg in enumerate(meta["segments"]):
            L = seg["n_obs"] * meta["BS"]
            last = si == len(meta["segments"]) - 1
            for rt in range(rt_count):
                psum = psum_pool.tile([P, 1024], F32, tag="mm")
                for (c0, c1, t, rb, kr, wc, start, stop) in seg["tasks"]:
                    lhsT = xt[rb:rb + kr, t * Nc + rt * P: t * Nc + (rt + 1) * P]
                    nc.tensor.matmul(
                        psum[:, c0:c1], lhsT, ws[rb:rb + kr, wc:wc + (c1 - c0)],
                        start=start, stop=stop)
                dst = out_sbs[rt][:, seg["out_base"]:seg["out_base"] + L]
                if ev % 2 == 0:
                    nc.scalar.copy(dst, psum[:, :L])
                else:
                    nc.vector.tensor_copy(out=dst, in_=psum[:, :L])
                ev += 1
                # flush finished output columns in ~0.8MB chunks; the store
                # stream shares SP's HWDGE FIFO, so it naturally yields to
                # the (critical) input stream queued ahead of it
                done = seg["out_base"] + L
                if done - flushed[rt] >= flush_cols or last:
                    nc.sync.dma_start(
                        out=out_d[rt * P:(rt + 1) * P, flushed[rt]:done],
                        in_=out_sbs[rt][:, flushed[rt]:done])
                    flushed[rt] = done
    nc.compile()
    return nc


def _host_tensors(meta, x2, weight):
    """Build per-core xt and shared ws host arrays (values only)."""
    BS = meta["BS"]
    Nc = meta["rows_per_core"]
    Ntot = Nc * N_CORES

    if x2.shape[0] < Ntot:
        x2 = np.concatenate(
            [x2, np.zeros((Ntot - x2.shape[0], x2.shape[1]), np.float32)], axis=0)

    # ws (shared): [128, ws_cols]
    ws = np.zeros((P, meta["ws_cols"]), np.float32)
    wsum = {}
    for (ob_ib, ks) in meta["wslots"].items():
        w = weight[ks[0]]
        for k in ks[1:]:
            w = w + weight[k]
        wsum[ob_ib] = np.ascontiguousarray(w, dtype=np.float32)
    for (wc, rb, uibs, seg_obs) in meta["ws_blocks"]:
        for r, ib in enumerate(uibs):
            row0 = rb + r * 64
            for j, ob in enumerate(seg_obs):
                w = wsum.get((ob, ib))
                if w is not None:
                    ws[row0:row0 + 64, wc + j * BS: wc + (j + 1) * BS] = w

    # xt per core: [128, n_tiles*Nc]; tile t covers cols [t*Nc, (t+1)*Nc)
    xt_all = []
    for c in range(N_CORES):
        xs = x2[c * Nc:(c + 1) * Nc]           # [Nc, F]
        xt = np.zeros((P, len(meta["xt_tiles"]) * Nc), np.float32)
        for t, entries in enumerate(meta["xt_tiles"]):
            for (rbase, ib) in entries:
                xt[rbase:rbase + 64, t * Nc:(t + 1) * Nc] = \
                    xs[:, ib * BS:(ib + 1) * BS].T
        xt_all.append(np.ascontiguousarray(xt.astype(NP_IN)))
    return xt_all, np.ascontiguousarray(ws.astype(NP_IN))


def kernel(**inputs):
    global LAST_RESULT
    x = np.asarray(inputs["x"], dtype=np.float32)
    weight = np.asarray(inputs["weight"], dtype=np.float32)
    bias = np.asarray(inputs["bias"], dtype=np.float32)
    out_idx = np.asarray(inputs["out_block_idx"]).astype(np.int64)
    in_idx = np.asarray(inputs["in_block_idx"]).astype(np.int64)

    B, S, F = x.shape
    N = B * S
    BS = weight.shape[1]
    OUT_F = bias.shape[0]
    x2 = np.ascontiguousarray(x.reshape(N, F))

    key = (N, F, OUT_F, BS, out_idx.tobytes(), in_idx.tobytes())
    if key not in _CACHE:
        meta = _build_schedule(N, F, OUT_F, BS, out_idx, in_idx)
        nc = _build_nc(meta)
        _CACHE[key] = (nc, meta)
    nc, meta = _CACHE[key]

    xt_all, ws = _host_tensors(meta, x2, weight)
    in_maps = [{"xt": xt_all[c], "ws": ws} for c in range(N_CORES)]
    res = bass_utils.run_bass_kernel_spmd(nc, in_maps, core_ids=list(range(N_CORES)))
    LAST_RESULT = res

    Nc = meta["rows_per_core"]
    dev = np.concatenate(
        [np.asarray(res.results[c]["out"]).astype(np.float32)
         for c in range(N_CORES)], axis=0)
    dev = dev[:N]  # drop row padding

    out = np.zeros((N, OUT_F), np.float32)
    for seg in meta["segments"]:
        b = seg["out_base"]
        for j, ob in enumerate(seg["obs"]):
            out[:, ob * BS:(ob + 1) * BS] = dev[:, b + j * BS: b + (j + 1) * BS]
    if bias.any():
        out += bias
    return out.reshape(B, S, OUT_F)


# revision 20
# speedup vs baseline: 1.7647x; 1.0121x over previous
"""Block-sparse linear kernel for Trainium2 (8 NeuronCores, Bass/Tile).

Computes out[n, ob*BS:(ob+1)*BS] += x[n, ib*BS:(ib+1)*BS] @ W[k]  for each
nonzero block k with indices (ob, ib), plus bias — data-parallel over the
flattened row dim N across 8 cores (weights/indices replicated).

Strategy (host-side schedule specialization from the index tensors):
  - Group input-blocks (ibs) into *families* with identical sets of
    output-blocks (obs).  Families whose obs-sets overlap are merged into
    *superfamilies* (zero-filled weight stacking keeps those correct).
  - Within a family, pair up ibs: a pair forms one K=128 stationary
    operand (the pair's two 64-feature slices of x, transposed host-side),
    streaming a [128, n_obs*64] stacked weight -> full PE utilization.
  - PSUM accumulates each superfamily-segment (<=16 obs = <=1024 f32 cols
    = 2 banks) over all its pairs/singles via matmul start/stop flags.
  - Output is laid out family-major (contiguous evictions); the host
    un-permutes output block columns and adds bias.
  - Matmuls run in float32r (TF32-like, ~1.5e-4 rel err, full PE rate).

The device kernel does: 2 input DMAs, matmul accumulation, PSUM->SBUF
evictions on ACT/DVE, 1 output DMA per 128-row tile.  All x transposition
and index logic happens on the host at schedule-build time.
"""

import os
import numpy as np
import ml_dtypes
from collections import defaultdict
from contextlib import ExitStack

from concourse import bass_utils, bacc, mybir
import concourse.tile as tile

N_CORES = 8
P = 128            # partitions / row-tile size
SEG_MAX_OBS = 16   # psum segment cap: 16 blocks * 64 = 1024 f32 = 2 banks
F32R = mybir.dt.float32r
F32 = mybir.dt.float32
BF16 = mybir.dt.bfloat16

# input dtype for the tensor engine: bf16 (default) halves input DMA and
# gets fast weight loads (~2.8e-3 rel err); f32r is TF32-like (~1.5e-4)
KDTYPE = os.environ.get("KDTYPE", "bf16")
DT_IN = BF16 if KDTYPE == "bf16" else F32R
NP_IN = ml_dtypes.bfloat16 if KDTYPE == "bf16" else np.float32
# output dtype: bf16 halves store traffic (adds ~2e-3 rounding, still far
# under the rel-err gate); f32 is exact
KOUT = os.environ.get("KOUT", "bf16")
DT_OUT = BF16 if KOUT == "bf16" else F32
NP_OUT = ml_dtypes.bfloat16 if KOUT == "bf16" else np.float32

# schedule-key -> (nc, meta) cache so repeated kernel() calls reuse the
# compiled module (and the NEFF cache underneath).
_CACHE = {}

# test harness introspection: last BassKernelResults
LAST_RESULT = None


def _build_schedule(N, F, OUT_F, BS, out_idx, in_idx):
    """Pure-index schedule: families, pairs, segments, layouts."""
    n_ib = F // BS
    n_ob = OUT_F // BS
    assert F % BS == 0 and OUT_F % BS == 0

    # (ob, ib) -> list of weight slots k (duplicates summed host-side)
    wslots = defaultdict(list)
    for k, (ob, ib) in enumerate(zip(out_idx, in_idx)):
        ob, ib = int(ob), int(ib)
        assert 0 <= ob < n_ob and 0 <= ib < n_ib
        wslots[(ob, ib)].append(k)

    obs_by_ib = defaultdict(set)
    for (ob, ib) in wslots:
        obs_by_ib[ib].add(ob)

    # families: ibs with identical obs sets
    fam_map = defaultdict(list)
    for ib in sorted(obs_by_ib):
        fam_map[frozenset(obs_by_ib[ib])].append(ib)
    families = [(sorted(obs), ibs) for obs, ibs in fam_map.items()]

    # union-find over obs to merge overlapping families into superfamilies
    parent = {}

    def find(a):
        while parent[a] != a:
            parent[a] = parent[parent[a]]
            a = parent[a]
        return a

    for obs, _ in families:
        for ob in obs:
            parent.setdefault(ob, ob)
        r = find(obs[0])
        for ob in obs[1:]:
            parent[find(ob)] = r
    sf_map = defaultdict(lambda: {"obs": set(), "fams": []})
    for obs, ibs in families:
        root = find(obs[0])
        sf_map[root]["obs"].update(obs)
        sf_map[root]["fams"].append((obs, ibs))
    superfams = sorted(sf_map.values(), key=lambda s: min(s["obs"]))

    # xt tile table: pairs (full K=128) and packed singles (K=64 halves)
    xt_tiles = []      # per tile: list of (rowbase, ib) entries
    unit_of = {}       # (fam_id, pair_idx) -> (tile_idx, rowbase, krows, ibs)
    singles = []       # deferred: (fam_key, ib)
    fam_units = defaultdict(list)   # fam key -> [(tile, rowbase, krows, ibs)]
    fam_id = 0
    fam_keys = {}
    for sf in superfams:
        for obs, ibs in sf["fams"]:
            key = fam_id
            fam_keys[key] = (tuple(obs), tuple(ibs))
            for i in range(0, len(ibs) - 1, 2):
                t = len(xt_tiles)
                xt_tiles.append([(0, ibs[i]), (64, ibs[i + 1])])
                fam_units[key].append((t, 0, 128, (ibs[i], ibs[i + 1])))
            if len(ibs) % 2:
                singles.append((key, ibs[-1]))
            fam_id += 1
    for j in range(0, len(singles), 2):
        t = len(xt_tiles)
        entries = [(0, singles[j][1])]
        fam_units[singles[j][0]].append((t, 0, 64, (singles[j][1],)))
        if j + 1 < len(singles):
            entries.append((64, singles[j + 1][1]))
            fam_units[singles[j + 1][0]].append((t, 64, 64, (singles[j + 1][1],)))
        xt_tiles.append(entries)

    # segments + ws layout + out layout
    # mm task: (psum_c0, psum_c1, tile, rowbase, krows, ws_c0, start, stop)
    segments = []   # per segment: dict(out_base, n_obs, obs, tasks)
    ws_blocks = []  # (ws_col, rowbase, ib_or_None, obs_list) for host fill
    ws_cols = 0
    out_cols = 0
    fid = 0
    for sf in superfams:
        sf_obs = sorted(sf["obs"])
        # family units of this superfamily, in deterministic order
        units = []
        base = fid
        for obs, ibs in sf["fams"]:
            units.append((fid, tuple(obs)))
            fid += 1
        for s0 in range(0, len(sf_obs), SEG_MAX_OBS):
            seg_obs = sf_obs[s0:s0 + SEG_MAX_OBS]
            L = len(seg_obs) * BS
            tasks = []
            all_units = []
            for key, fobs in units:
                for (t, rb, kr, uibs) in fam_units[key]:
                    all_units.append((t, rb, kr, uibs))
            seg_ws0 = ws_cols
            unit_ws = []
            unit_tiles = []
            for ui, (t, rb, kr, uibs) in enumerate(all_units):
                wc = ws_cols
                ws_blocks.append((wc, rb, uibs, seg_obs))
                unit_ws.append((wc, wc + L))
                unit_tiles.append(t)
                for c0 in range(0, L, 512):
                    c1 = min(c0 + 512, L)
                    tasks.append((c0, c1, t, rb, kr, wc + c0,
                                  ui == 0, ui == len(all_units) - 1))
                ws_cols += L
            segments.append({"out_base": out_cols, "n_obs": len(seg_obs),
                             "obs": seg_obs, "tasks": tasks,
                             "ws0": seg_ws0, "ws1": ws_cols,
                             "unit_ws": unit_ws, "unit_tiles": unit_tiles,
                             "tiles": sorted({tk[2] for tk in tasks})})
            out_cols += L

    n_pad = (-N) % (N_CORES * P)
    rows_per_core = (N + n_pad) // N_CORES
    rt_count = rows_per_core // P

    # input-DMA load plan in consumption order: ("ws"|"xt", c0, c1).
    # xt entries are tile-index ranges; first segment's ws goes per-unit so
    # the very first matmul only waits on a small chunk.
    load_plan = []
    seen_tiles = set()

    def add_tiles(tiles):
        new_t = [t for t in tiles if t not in seen_tiles]
        seen_tiles.update(new_t)
        i = 0
        while i < len(new_t):   # merge consecutive tile indices into ranges
            j = i
            while j + 1 < len(new_t) and new_t[j + 1] == new_t[j] + 1:
                j += 1
            load_plan.append(("xt", new_t[i], new_t[j] + 1))
            i = j + 1

    for si, seg in enumerate(segments):
        if si == 0:
            # finest interleave: each unit's ws chunk followed by its xt
            # tile, so the first matmul waits on ~0.7MB only
            for (a, b), t in zip(seg["unit_ws"], seg["unit_tiles"]):
                load_plan.append(("ws", a, b))
                add_tiles([t])
        else:
            load_plan.append(("ws", seg["ws0"], seg["ws1"]))
            add_tiles(seg["tiles"])

    return {
        "N": N, "F": F, "OUT_F": OUT_F, "BS": BS,
        "wslots": dict(wslots),
        "xt_tiles": xt_tiles,
        "ws_blocks": ws_blocks, "ws_cols": ws_cols,
        "segments": segments, "out_cols": out_cols,
        "rows_per_core": rows_per_core, "rt_count": rt_count,
        "load_plan": load_plan,
    }


def _build_nc(meta):
    """Emit the Bass/Tile module for a schedule (value-independent)."""
    Nc = meta["rows_per_core"]
    XTC = len(meta["xt_tiles"]) * Nc
    WSC = meta["ws_cols"]
    OUTC = meta["out_cols"]
    rt_count = meta["rt_count"]

    nc = bacc.Bacc("TRN2", target_bir_lowering=False, debug=False)
    xt_d = nc.dram_tensor("xt", [P, XTC], DT_IN, kind="ExternalInput")
    ws_d = nc.dram_tensor("ws", [P, WSC], DT_IN, kind="ExternalInput")
    out_d = nc.dram_tensor("out", [Nc, OUTC], DT_OUT, kind="ExternalOutput")

    n_warm = int(os.environ.get("KWARM", "8"))
    flush_cols = int(os.environ.get("KFLUSH", "800"))

    with tile.TileContext(nc) as tc, ExitStack() as ctx:
        xt_pool = ctx.enter_context(tc.tile_pool(name="xt", bufs=1))
        warm_pool = ctx.enter_context(tc.tile_pool(name="wm", bufs=1))
        ws_pool = ctx.enter_context(tc.tile_pool(name="ws", bufs=1))
        psum_pool = ctx.enter_context(tc.tile_pool(name="ps", bufs=4, space="PSUM"))
        out_pool = ctx.enter_context(tc.tile_pool(name="ot", bufs=1))

        xt = xt_pool.tile([P, XTC], DT_IN)
        ws = ws_pool.tile([P, WSC], DT_IN)

        # PE warm-up on an uninitialized scratch tile: no DMA dependency, so
        # the dummies run as soon as the PE preamble finishes and flip HAM
        # to 8/8 before the first real matmul.  Values are garbage; the psum
        # scratch is never read.
        if n_warm:
            wsb = warm_pool.tile([P, 512], DT_IN)
            nc.gpsimd.memset(wsb[:].bitcast(F32), 0)
            wps = psum_pool.tile([P, 1024], F32, tag="mm")
            for _ in range(n_warm):
                nc.tensor.matmul(wps[:, :512], wsb[:, :P], wsb[:, :512],
                                 start=True, stop=True)

        # chunked input DMAs in first-use order so matmuls start early; the
        # first two (latency-critical) chunks go down ACT's parallel HWDGE
        # ring while SP streams the rest.
        for li, (kind, a, b) in enumerate(meta["load_plan"]):
            eng = nc.scalar if li < 2 else nc.sync
            if kind == "ws":
                eng.dma_start(out=ws[:, a:b], in_=ws_d[:, a:b])
            else:
                eng.dma_start(out=xt[:, a * Nc:b * Nc], in_=xt_d[:, a * Nc:b * Nc])

        # segment-outer / row-tile-inner: each segment's data is consumed
        # for all row tiles right after it lands, so the PE runs dense and
        # stays ahead of the input stream instead of idling through it.
        out_sbs = [out_pool.tile([P, OUTC], DT_OUT, name=f"osb{r}", tag=f"osb{r}")
                   for r in range(rt_count)]
        flushed = [0] * rt_count
        ev = 0
        for si, seg in enumerate(meta["segments"]):
            L = seg["n_obs"] * meta["BS"]
            last = si == len(meta["segments"]) - 1
            for rt in range(rt_count):
                psum = psum_pool.tile([P, 1024], F32, tag="mm")
                for (c0, c1, t, rb, kr, wc, start, stop) in seg["tasks"]:
                    lhsT = xt[rb:rb + kr, t * Nc + rt * P: t * Nc + (rt + 1) * P]
                    nc.tensor.matmul(
                        psum[:, c0:c1], lhsT, ws[rb:rb + kr, wc:wc + (c1 - c0)],
                        start=start, stop=stop)
                dst = out_sbs[rt][:, seg["out_base"]:seg["out_base"] + L]
                if ev % 2 == 0:
                    nc.scalar.copy(dst, psum[:, :L])
                else:
                    nc.vector.tensor_copy(out=dst, in_=psum[:, :L])
                ev += 1
                # flush finished output columns in ~0.8MB chunks; the store
                # stream shares SP's HWDGE FIFO, so it naturally yields to
                # the (critical) input stream queued ahead of it
                done = seg["out_base"] + L
                if done - flushed[rt] >= flush_cols or last:
                    nc.sync.dma_start(
                        out=out_d[rt * P:(rt + 1) * P, flushed[rt]:done],
                        in_=out_sbs[rt][:, flushed[rt]:done])
                    flushed[rt] = done
    nc.compile()
    return nc


def _host_tensors(meta, x2, weight):
    """Build per-core xt and shared ws host arrays (values only)."""
    BS = meta["BS"]
    Nc = meta["rows_per_core"]
    Ntot = Nc * N_CORES

    if x2.shape[0] < Ntot:
        x2 = np.concatenate(
            [x2, np.zeros((Ntot - x2.shape[0], x2.shape[1]), np.float32)], axis=0)

    # ws (shared): [128, ws_cols]
    ws = np.zeros((P, meta["ws_cols"]), np.float32)
    wsum = {}
    for (ob_ib, ks) in meta["wslots"].items():
        w = weight[ks[0]]
        for k in ks[1:]:
            w = w + weight[k]
        wsum[ob_ib] = np.ascontiguousarray(w, dtype=np.float32)
    for (wc, rb, uibs, seg_obs) in meta["ws_blocks"]:
        for r, ib in enumerate(uibs):
            row0 = rb + r * 64
            for j, ob in enumerate(seg_obs):
                w = wsum.get((ob, ib))
                if w is not None:
                    ws[row0:row0 + 64, wc + j * BS: wc + (j + 1) * BS] = w

    # xt per core: [128, n_tiles*Nc]; tile t covers cols [t*Nc, (t+1)*Nc)
    xt_all = []
    for c in range(N_CORES):
        xs = x2[c * Nc:(c + 1) * Nc]           # [Nc, F]
        xt = np.zeros((P, len(meta["xt_tiles"]) * Nc), np.float32)
        for t, entries in enumerate(meta["xt_tiles"]):
            for (rbase, ib) in entries:
                xt[rbase:rbase + 64, t * Nc:(t + 1) * Nc] = \
                    xs[:, ib * BS:(ib + 1) * BS].T
        xt_all.append(np.ascontiguousarray(xt.astype(NP_IN)))
    return xt_all, np.ascontiguousarray(ws.astype(NP_IN))


def kernel(**inputs):
    global LAST_RESULT
    x = np.asarray(inputs["x"], dtype=np.float32)
    weight = np.asarray(inputs["weight"], dtype=np.float32)
    bias = np.asarray(inputs["bias"], dtype=np.float32)
    out_idx = np.asarray(inputs["out_block_idx"]).astype(np.int64)
    in_idx = np.asarray(inputs["in_block_idx"]).astype(np.int64)

    B, S, F = x.shape
    N = B * S
    BS = weight.shape[1]
    OUT_F = bias.shape[0]
    x2 = np.ascontiguousarray(x.reshape(N, F))

    key = (N, F, OUT_F, BS, out_idx.tobytes(), in_idx.tobytes())
    if key not in _CACHE:
        meta = _build_schedule(N, F, OUT_F, BS, out_idx, in_idx)
        nc = _build_nc(meta)
        _CACHE[key] = (nc, meta)
    nc, meta = _CACHE[key]

    xt_all, ws = _host_tensors(meta, x2, weight)
    in_maps = [{"xt": xt_all[c], "ws": ws} for c in range(N_CORES)]
    res = bass_utils.run_bass_kernel_spmd(nc, in_maps, core_ids=list(range(N_CORES)))
    LAST_RESULT = res

    Nc = meta["rows_per_core"]
    dev = np.concatenate(
        [np.asarray(res.results[c]["out"]).astype(np.float32)
         for c in range(N_CORES)], axis=0)
    dev = dev[:N]  # drop row padding

    out = np.zeros((N, OUT_F), np.float32)
    for seg in meta["segments"]:
        b = seg["out_base"]
        for j, ob in enumerate(seg["obs"]):
            out[:, ob * BS:(ob + 1) * BS] = dev[:, b + j * BS: b + (j + 1) * BS]
    if bias.any():
        out += bias
    return out.reshape(B, S, OUT_F)


# revision 21
# speedup vs baseline: 1.8189x; 1.0307x over previous
"""Block-sparse linear kernel for Trainium2 (8 NeuronCores, Bass/Tile).

Computes out[n, ob*BS:(ob+1)*BS] += x[n, ib*BS:(ib+1)*BS] @ W[k]  for each
nonzero block k with indices (ob, ib), plus bias — data-parallel over the
flattened row dim N across 8 cores (weights/indices replicated).

Strategy (host-side schedule specialization from the index tensors):
  - Group input-blocks (ibs) into *families* with identical sets of
    output-blocks (obs).  Families whose obs-sets overlap are merged into
    *superfamilies* (zero-filled weight stacking keeps those correct).
  - Within a family, pair up ibs: a pair forms one K=128 stationary
    operand (the pair's two 64-feature slices of x, transposed host-side),
    streaming a [128, n_obs*64] stacked weight -> full PE utilization.
  - PSUM accumulates each superfamily-segment (<=16 obs = <=1024 f32 cols
    = 2 banks) over all its pairs/singles via matmul start/stop flags.
  - Output is laid out family-major (contiguous evictions); the host
    un-permutes output block columns and adds bias.
  - Matmuls run in float32r (TF32-like, ~1.5e-4 rel err, full PE rate).

The device kernel does: 2 input DMAs, matmul accumulation, PSUM->SBUF
evictions on ACT/DVE, 1 output DMA per 128-row tile.  All x transposition
and index logic happens on the host at schedule-build time.
"""

import os
import numpy as np
import ml_dtypes
from collections import defaultdict
from contextlib import ExitStack

from concourse import bass_utils, bacc, mybir
import concourse.tile as tile

N_CORES = 8
P = 128            # partitions / row-tile size
SEG_MAX_OBS = 16   # psum segment cap: 16 blocks * 64 = 1024 f32 = 2 banks
F32R = mybir.dt.float32r
F32 = mybir.dt.float32
BF16 = mybir.dt.bfloat16

# input dtype for the tensor engine: bf16 (default) halves input DMA and
# gets fast weight loads (~2.8e-3 rel err); f32r is TF32-like (~1.5e-4)
KDTYPE = os.environ.get("KDTYPE", "bf16")
DT_IN = BF16 if KDTYPE == "bf16" else F32R
NP_IN = ml_dtypes.bfloat16 if KDTYPE == "bf16" else np.float32
# output dtype: bf16 halves store traffic (adds ~2e-3 rounding, still far
# under the rel-err gate); f32 is exact
KOUT = os.environ.get("KOUT", "bf16")
DT_OUT = BF16 if KOUT == "bf16" else F32
NP_OUT = ml_dtypes.bfloat16 if KOUT == "bf16" else np.float32

# schedule-key -> (nc, meta) cache so repeated kernel() calls reuse the
# compiled module (and the NEFF cache underneath).
_CACHE = {}

# test harness introspection: last BassKernelResults
LAST_RESULT = None


def _build_schedule(N, F, OUT_F, BS, out_idx, in_idx):
    """Pure-index schedule: families, pairs, segments, layouts."""
    n_ib = F // BS
    n_ob = OUT_F // BS
    assert F % BS == 0 and OUT_F % BS == 0

    # (ob, ib) -> list of weight slots k (duplicates summed host-side)
    wslots = defaultdict(list)
    for k, (ob, ib) in enumerate(zip(out_idx, in_idx)):
        ob, ib = int(ob), int(ib)
        assert 0 <= ob < n_ob and 0 <= ib < n_ib
        wslots[(ob, ib)].append(k)

    obs_by_ib = defaultdict(set)
    for (ob, ib) in wslots:
        obs_by_ib[ib].add(ob)

    # families: ibs with identical obs sets
    fam_map = defaultdict(list)
    for ib in sorted(obs_by_ib):
        fam_map[frozenset(obs_by_ib[ib])].append(ib)
    families = [(sorted(obs), ibs) for obs, ibs in fam_map.items()]

    # union-find over obs to merge overlapping families into superfamilies
    parent = {}

    def find(a):
        while parent[a] != a:
            parent[a] = parent[parent[a]]
            a = parent[a]
        return a

    for obs, _ in families:
        for ob in obs:
            parent.setdefault(ob, ob)
        r = find(obs[0])
        for ob in obs[1:]:
            parent[find(ob)] = r
    sf_map = defaultdict(lambda: {"obs": set(), "fams": []})
    for obs, ibs in families:
        root = find(obs[0])
        sf_map[root]["obs"].update(obs)
        sf_map[root]["fams"].append((obs, ibs))
    superfams = sorted(sf_map.values(), key=lambda s: min(s["obs"]))

    # xt tile table: pairs (full K=128) and packed singles (K=64 halves)
    xt_tiles = []      # per tile: list of (rowbase, ib) entries
    unit_of = {}       # (fam_id, pair_idx) -> (tile_idx, rowbase, krows, ibs)
    singles = []       # deferred: (fam_key, ib)
    fam_units = defaultdict(list)   # fam key -> [(tile, rowbase, krows, ibs)]
    fam_id = 0
    fam_keys = {}
    for sf in superfams:
        for obs, ibs in sf["fams"]:
            key = fam_id
            fam_keys[key] = (tuple(obs), tuple(ibs))
            for i in range(0, len(ibs) - 1, 2):
                t = len(xt_tiles)
                xt_tiles.append([(0, ibs[i]), (64, ibs[i + 1])])
                fam_units[key].append((t, 0, 128, (ibs[i], ibs[i + 1])))
            if len(ibs) % 2:
                singles.append((key, ibs[-1]))
            fam_id += 1
    for j in range(0, len(singles), 2):
        t = len(xt_tiles)
        entries = [(0, singles[j][1])]
        fam_units[singles[j][0]].append((t, 0, 64, (singles[j][1],)))
        if j + 1 < len(singles):
            entries.append((64, singles[j + 1][1]))
            fam_units[singles[j + 1][0]].append((t, 64, 64, (singles[j + 1][1],)))
        xt_tiles.append(entries)

    # segments + ws layout + out layout
    # mm task: (psum_c0, psum_c1, tile, rowbase, krows, ws_c0, start, stop)
    segments = []   # per segment: dict(out_base, n_obs, obs, tasks)
    ws_blocks = []  # (ws_col, rowbase, ib_or_None, obs_list) for host fill
    ws_cols = 0
    out_cols = 0
    fid = 0
    for sf in superfams:
        sf_obs = sorted(sf["obs"])
        # family units of this superfamily, in deterministic order
        units = []
        base = fid
        for obs, ibs in sf["fams"]:
            units.append((fid, tuple(obs)))
            fid += 1
        for s0 in range(0, len(sf_obs), SEG_MAX_OBS):
            seg_obs = sf_obs[s0:s0 + SEG_MAX_OBS]
            L = len(seg_obs) * BS
            tasks = []
            all_units = []
            for key, fobs in units:
                for (t, rb, kr, uibs) in fam_units[key]:
                    all_units.append((t, rb, kr, uibs))
            seg_ws0 = ws_cols
            unit_ws = []
            unit_tiles = []
            for ui, (t, rb, kr, uibs) in enumerate(all_units):
                wc = ws_cols
                ws_blocks.append((wc, rb, uibs, seg_obs))
                unit_ws.append((wc, wc + L))
                unit_tiles.append(t)
                for c0 in range(0, L, 512):
                    c1 = min(c0 + 512, L)
                    tasks.append((c0, c1, t, rb, kr, wc + c0,
                                  ui == 0, ui == len(all_units) - 1))
                ws_cols += L
            segments.append({"out_base": out_cols, "n_obs": len(seg_obs),
                             "obs": seg_obs, "tasks": tasks,
                             "ws0": seg_ws0, "ws1": ws_cols,
                             "unit_ws": unit_ws, "unit_tiles": unit_tiles,
                             "tiles": sorted({tk[2] for tk in tasks})})
            out_cols += L

    n_pad = (-N) % (N_CORES * P)
    rows_per_core = (N + n_pad) // N_CORES
    rt_count = rows_per_core // P

    # input-DMA load plan in consumption order: ("ws"|"xt", c0, c1).
    # xt entries are tile-index ranges; first segment's ws goes per-unit so
    # the very first matmul only waits on a small chunk.
    load_plan = []
    seen_tiles = set()

    def add_tiles(tiles):
        new_t = [t for t in tiles if t not in seen_tiles]
        seen_tiles.update(new_t)
        i = 0
        while i < len(new_t):   # merge consecutive tile indices into ranges
            j = i
            while j + 1 < len(new_t) and new_t[j + 1] == new_t[j] + 1:
                j += 1
            load_plan.append(("xt", new_t[i], new_t[j] + 1))
            i = j + 1

    for si, seg in enumerate(segments):
        if si == 0:
            # finest interleave: each unit's ws chunk followed by its xt
            # tile, so the first matmul waits on ~0.7MB only
            for (a, b), t in zip(seg["unit_ws"], seg["unit_tiles"]):
                load_plan.append(("ws", a, b))
                add_tiles([t])
        else:
            load_plan.append(("ws", seg["ws0"], seg["ws1"]))
            add_tiles(seg["tiles"])

    return {
        "N": N, "F": F, "OUT_F": OUT_F, "BS": BS,
        "wslots": dict(wslots),
        "xt_tiles": xt_tiles,
        "ws_blocks": ws_blocks, "ws_cols": ws_cols,
        "segments": segments, "out_cols": out_cols,
        "rows_per_core": rows_per_core, "rt_count": rt_count,
        "load_plan": load_plan,
    }


def _build_nc(meta):
    """Emit the Bass/Tile module for a schedule (value-independent)."""
    Nc = meta["rows_per_core"]
    XTC = len(meta["xt_tiles"]) * Nc
    WSC = meta["ws_cols"]
    OUTC = meta["out_cols"]
    rt_count = meta["rt_count"]

    nc = bacc.Bacc("TRN2", target_bir_lowering=False, debug=False)
    xt_d = nc.dram_tensor("xt", [P, XTC], DT_IN, kind="ExternalInput")
    ws_d = nc.dram_tensor("ws", [P, WSC], DT_IN, kind="ExternalInput")
    out_d = nc.dram_tensor("out", [Nc, OUTC], DT_OUT, kind="ExternalOutput")

    n_warm = int(os.environ.get("KWARM", "8"))
    flush_cols = int(os.environ.get("KFLUSH", "800"))

    with tile.TileContext(nc) as tc, ExitStack() as ctx:
        xt_pool = ctx.enter_context(tc.tile_pool(name="xt", bufs=1))
        warm_pool = ctx.enter_context(tc.tile_pool(name="wm", bufs=1))
        ws_pool = ctx.enter_context(tc.tile_pool(name="ws", bufs=1))
        psum_pool = ctx.enter_context(tc.tile_pool(name="ps", bufs=4, space="PSUM"))
        out_pool = ctx.enter_context(tc.tile_pool(name="ot", bufs=1))

        xt = xt_pool.tile([P, XTC], DT_IN)
        ws = ws_pool.tile([P, WSC], DT_IN)

        # PE warm-up on an uninitialized scratch tile: no DMA dependency, so
        # the dummies run as soon as the PE preamble finishes and flip HAM
        # to 8/8 before the first real matmul.  Values are garbage; the psum
        # scratch is never read.
        if n_warm:
            wsb = warm_pool.tile([P, 512], DT_IN)
            nc.gpsimd.memset(wsb[:].bitcast(F32), 0)
            wps = psum_pool.tile([P, 1024], F32, tag="mm")
            for _ in range(n_warm):
                nc.tensor.matmul(wps[:, :512], wsb[:, :P], wsb[:, :512],
                                 start=True, stop=True)

        # chunked input DMAs in first-use order so matmuls start early; the
        # first two (latency-critical) chunks go down ACT's parallel HWDGE
        # ring while SP streams the rest.
        for li, (kind, a, b) in enumerate(meta["load_plan"]):
            if kind == "ws":
                nc.sync.dma_start(out=ws[:, a:b], in_=ws_d[:, a:b])
            else:
                nc.sync.dma_start(out=xt[:, a * Nc:b * Nc], in_=xt_d[:, a * Nc:b * Nc])

        # segment-outer / row-tile-inner: each segment's data is consumed
        # for all row tiles right after it lands, so the PE runs dense and
        # stays ahead of the input stream instead of idling through it.
        out_sbs = [out_pool.tile([P, OUTC], DT_OUT, name=f"osb{r}", tag=f"osb{r}")
                   for r in range(rt_count)]
        flushed = [0] * rt_count
        ev = 0
        for si, seg in enumerate(meta["segments"]):
            L = seg["n_obs"] * meta["BS"]
            last = si == len(meta["segments"]) - 1
            for rt in range(rt_count):
                psum = psum_pool.tile([P, 1024], F32, tag="mm")
                for (c0, c1, t, rb, kr, wc, start, stop) in seg["tasks"]:
                    lhsT = xt[rb:rb + kr, t * Nc + rt * P: t * Nc + (rt + 1) * P]
                    nc.tensor.matmul(
                        psum[:, c0:c1], lhsT, ws[rb:rb + kr, wc:wc + (c1 - c0)],
                        start=start, stop=stop)
                dst = out_sbs[rt][:, seg["out_base"]:seg["out_base"] + L]
                if ev % 2 == 0:
                    nc.scalar.copy(dst, psum[:, :L])
                else:
                    nc.vector.tensor_copy(out=dst, in_=psum[:, :L])
                ev += 1
                # flush finished output columns in ~0.8MB chunks; the store
                # stream shares SP's HWDGE FIFO, so it naturally yields to
                # the (critical) input stream queued ahead of it
                done = seg["out_base"] + L
                if done - flushed[rt] >= flush_cols or last:
                    nc.sync.dma_start(
                        out=out_d[rt * P:(rt + 1) * P, flushed[rt]:done],
                        in_=out_sbs[rt][:, flushed[rt]:done])
                    flushed[rt] = done
    nc.compile()
    return nc


def _host_tensors(meta, x2, weight):
    """Build per-core xt and shared ws host arrays (values only)."""
    BS = meta["BS"]
    Nc = meta["rows_per_core"]
    Ntot = Nc * N_CORES

    if x2.shape[0] < Ntot:
        x2 = np.concatenate(
            [x2, np.zeros((Ntot - x2.shape[0], x2.shape[1]), np.float32)], axis=0)

    # ws (shared): [128, ws_cols]
    ws = np.zeros((P, meta["ws_cols"]), np.float32)
    wsum = {}
    for (ob_ib, ks) in meta["wslots"].items():
        w = weight[ks[0]]
        for k in ks[1:]:
            w = w + weight[k]
        wsum[ob_ib] = np.ascontiguousarray(w, dtype=np.float32)
    for (wc, rb, uibs, seg_obs) in meta["ws_blocks"]:
        for r, ib in enumerate(uibs):
            row0 = rb + r * 64
            for j, ob in enumerate(seg_obs):
                w = wsum.get((ob, ib))
                if w is not None:
                    ws[row0:row0 + 64, wc + j * BS: wc + (j + 1) * BS] = w

    # xt per core: [128, n_tiles*Nc]; tile t covers cols [t*Nc, (t+1)*Nc)
    xt_all = []
    for c in range(N_CORES):
        xs = x2[c * Nc:(c + 1) * Nc]           # [Nc, F]
        xt = np.zeros((P, len(meta["xt_tiles"]) * Nc), np.float32)
        for t, entries in enumerate(meta["xt_tiles"]):
            for (rbase, ib) in entries:
                xt[rbase:rbase + 64, t * Nc:(t + 1) * Nc] = \
                    xs[:, ib * BS:(ib + 1) * BS].T
        xt_all.append(np.ascontiguousarray(xt.astype(NP_IN)))
    return xt_all, np.ascontiguousarray(ws.astype(NP_IN))


def kernel(**inputs):
    global LAST_RESULT
    x = np.asarray(inputs["x"], dtype=np.float32)
    weight = np.asarray(inputs["weight"], dtype=np.float32)
    bias = np.asarray(inputs["bias"], dtype=np.float32)
    out_idx = np.asarray(inputs["out_block_idx"]).astype(np.int64)
    in_idx = np.asarray(inputs["in_block_idx"]).astype(np.int64)

    B, S, F = x.shape
    N = B * S
    BS = weight.shape[1]
    OUT_F = bias.shape[0]
    x2 = np.ascontiguousarray(x.reshape(N, F))

    key = (N, F, OUT_F, BS, out_idx.tobytes(), in_idx.tobytes())
    if key not in _CACHE:
        meta = _build_schedule(N, F, OUT_F, BS, out_idx, in_idx)
        nc = _build_nc(meta)
        _CACHE[key] = (nc, meta)
    nc, meta = _CACHE[key]

    xt_all, ws = _host_tensors(meta, x2, weight)
    in_maps = [{"xt": xt_all[c], "ws": ws} for c in range(N_CORES)]
    res = bass_utils.run_bass_kernel_spmd(nc, in_maps, core_ids=list(range(N_CORES)))
    LAST_RESULT = res

    Nc = meta["rows_per_core"]
    dev = np.concatenate(
        [np.asarray(res.results[c]["out"]).astype(np.float32)
         for c in range(N_CORES)], axis=0)
    dev = dev[:N]  # drop row padding

    out = np.zeros((N, OUT_F), np.float32)
    for seg in meta["segments"]:
        b = seg["out_base"]
        for j, ob in enumerate(seg["obs"]):
            out[:, ob * BS:(ob + 1) * BS] = dev[:, b + j * BS: b + (j + 1) * BS]
    if bias.any():
        out += bias
    return out.reshape(B, S, OUT_F)


# revision 23
# speedup vs baseline: 1.8777x; 1.0323x over previous
"""Block-sparse linear kernel for Trainium2 (8 NeuronCores, Bass/Tile).

Computes out[n, ob*BS:(ob+1)*BS] += x[n, ib*BS:(ib+1)*BS] @ W[k]  for each
nonzero block k with indices (ob, ib), plus bias — data-parallel over the
flattened row dim N across 8 cores (weights/indices replicated).

Strategy (host-side schedule specialization from the index tensors):
  - Group input-blocks (ibs) into *families* with identical sets of
    output-blocks (obs).  Families whose obs-sets overlap are merged into
    *superfamilies* (zero-filled weight stacking keeps those correct).
  - Within a family, pair up ibs: a pair forms one K=128 stationary
    operand (the pair's two 64-feature slices of x, transposed host-side),
    streaming a [128, n_obs*64] stacked weight -> full PE utilization.
  - PSUM accumulates each superfamily-segment (<=16 obs = <=1024 f32 cols
    = 2 banks) over all its pairs/singles via matmul start/stop flags.
  - Output is laid out family-major (contiguous evictions); the host
    un-permutes output block columns and adds bias.
  - Matmuls run in float32r (TF32-like, ~1.5e-4 rel err, full PE rate).

The device kernel does: 2 input DMAs, matmul accumulation, PSUM->SBUF
evictions on ACT/DVE, 1 output DMA per 128-row tile.  All x transposition
and index logic happens on the host at schedule-build time.
"""

import os
import numpy as np
import ml_dtypes
from collections import defaultdict
from contextlib import ExitStack

from concourse import bass_utils, bacc, mybir
import concourse.tile as tile

N_CORES = 8
P = 128            # partitions / row-tile size
SEG_MAX_OBS = 16   # psum segment cap: 16 blocks * 64 = 1024 f32 = 2 banks
F32R = mybir.dt.float32r
F32 = mybir.dt.float32
BF16 = mybir.dt.bfloat16

# input dtype for the tensor engine: bf16 (default) halves input DMA and
# gets fast weight loads (~2.8e-3 rel err); f32r is TF32-like (~1.5e-4)
KDTYPE = os.environ.get("KDTYPE", "bf16")
DT_IN = BF16 if KDTYPE == "bf16" else F32R
NP_IN = ml_dtypes.bfloat16 if KDTYPE == "bf16" else np.float32
# output dtype: bf16 halves store traffic (adds ~2e-3 rounding, still far
# under the rel-err gate); f32 is exact
KOUT = os.environ.get("KOUT", "bf16")
DT_OUT = BF16 if KOUT == "bf16" else F32
NP_OUT = ml_dtypes.bfloat16 if KOUT == "bf16" else np.float32

# schedule-key -> (nc, meta) cache so repeated kernel() calls reuse the
# compiled module (and the NEFF cache underneath).
_CACHE = {}

# test harness introspection: last BassKernelResults
LAST_RESULT = None


def _build_schedule(N, F, OUT_F, BS, out_idx, in_idx):
    """Pure-index schedule: families, pairs, segments, layouts."""
    n_ib = F // BS
    n_ob = OUT_F // BS
    assert F % BS == 0 and OUT_F % BS == 0

    # (ob, ib) -> list of weight slots k (duplicates summed host-side)
    wslots = defaultdict(list)
    for k, (ob, ib) in enumerate(zip(out_idx, in_idx)):
        ob, ib = int(ob), int(ib)
        assert 0 <= ob < n_ob and 0 <= ib < n_ib
        wslots[(ob, ib)].append(k)

    obs_by_ib = defaultdict(set)
    for (ob, ib) in wslots:
        obs_by_ib[ib].add(ob)

    # families: ibs with identical obs sets
    fam_map = defaultdict(list)
    for ib in sorted(obs_by_ib):
        fam_map[frozenset(obs_by_ib[ib])].append(ib)
    families = [(sorted(obs), ibs) for obs, ibs in fam_map.items()]

    # union-find over obs to merge overlapping families into superfamilies
    parent = {}

    def find(a):
        while parent[a] != a:
            parent[a] = parent[parent[a]]
            a = parent[a]
        return a

    for obs, _ in families:
        for ob in obs:
            parent.setdefault(ob, ob)
        r = find(obs[0])
        for ob in obs[1:]:
            parent[find(ob)] = r
    sf_map = defaultdict(lambda: {"obs": set(), "fams": []})
    for obs, ibs in families:
        root = find(obs[0])
        sf_map[root]["obs"].update(obs)
        sf_map[root]["fams"].append((obs, ibs))
    superfams = sorted(sf_map.values(), key=lambda s: min(s["obs"]))

    # xt tile table: pairs (full K=128) and packed singles (K=64 halves)
    xt_tiles = []      # per tile: list of (rowbase, ib) entries
    unit_of = {}       # (fam_id, pair_idx) -> (tile_idx, rowbase, krows, ibs)
    singles = []       # deferred: (fam_key, ib)
    fam_units = defaultdict(list)   # fam key -> [(tile, rowbase, krows, ibs)]
    fam_id = 0
    fam_keys = {}
    for sf in superfams:
        for obs, ibs in sf["fams"]:
            key = fam_id
            fam_keys[key] = (tuple(obs), tuple(ibs))
            for i in range(0, len(ibs) - 1, 2):
                t = len(xt_tiles)
                xt_tiles.append([(0, ibs[i]), (64, ibs[i + 1])])
                fam_units[key].append((t, 0, 128, (ibs[i], ibs[i + 1])))
            if len(ibs) % 2:
                singles.append((key, ibs[-1]))
            fam_id += 1
    for j in range(0, len(singles), 2):
        t = len(xt_tiles)
        entries = [(0, singles[j][1])]
        fam_units[singles[j][0]].append((t, 0, 64, (singles[j][1],)))
        if j + 1 < len(singles):
            entries.append((64, singles[j + 1][1]))
            fam_units[singles[j + 1][0]].append((t, 64, 64, (singles[j + 1][1],)))
        xt_tiles.append(entries)

    n_pad = (-N) % (N_CORES * P)
    rows_per_core = (N + n_pad) // N_CORES
    rt_count = rows_per_core // P
    Nc_ref = [rows_per_core]

    # segments + combined-input layout + out layout.
    # The device reads ONE input tensor "inp" [128, IN_COLS] laid out in
    # exact consumption order: per segment, each unit's stacked-weight block
    # followed by that unit's xt tile (if not yet placed).  One sequential
    # DMA stream then delivers data just-in-time with maximal efficiency.
    # mm task: (psum_c0, psum_c1, lhs_col, rowbase, krows, rhs_col)
    segments = []
    in_blocks = []  # (col, kind, payload): ("w", rowbase, uibs, seg_obs) | ("x", tile_idx)
    xt_off = {}     # tile idx -> col offset in inp
    in_cols = 0
    out_cols = 0
    cuts = []       # load-plan cut points (col indices), first chunk small
    fid = 0
    for sfi, sf in enumerate(superfams):
        sf_obs = sorted(sf["obs"])
        units = []
        for obs, ibs in sf["fams"]:
            units.append((fid, tuple(obs)))
            fid += 1
        for s0 in range(0, len(sf_obs), SEG_MAX_OBS):
            seg_obs = sf_obs[s0:s0 + SEG_MAX_OBS]
            L = len(seg_obs) * BS
            tasks = []
            all_units = []
            for key, fobs in units:
                for (t, rb, kr, uibs) in fam_units[key]:
                    all_units.append((t, rb, kr, uibs))
            for ui, (t, rb, kr, uibs) in enumerate(all_units):
                wc = in_cols
                in_blocks.append((wc, "w", rb, uibs, seg_obs))
                in_cols += L
                if t not in xt_off:
                    xt_off[t] = in_cols
                    in_blocks.append((in_cols, "x", t, None, None))
                    in_cols += Nc_ref[0]
                for c0 in range(0, L, 512):
                    c1 = min(c0 + 512, L)
                    tasks.append((c0, c1, xt_off[t], rb, kr, wc + c0,
                                  ui == 0, ui == len(all_units) - 1))
                if len(cuts) == 0 and len(segments) == 0 and ui == 0:
                    cuts.append(in_cols)   # first chunk: unit0 (+ its xt)
            segments.append({"out_base": out_cols, "n_obs": len(seg_obs),
                             "obs": seg_obs, "tasks": tasks})
            out_cols += L
    cuts.append(in_cols)

    # chunk the input stream at ~CHUNK_COLS boundaries between the cuts
    CHUNK_COLS = 3400
    block_edges = sorted({b[0] for b in in_blocks} | {in_cols})
    load_plan = []
    prev = 0
    for edge in block_edges[1:]:
        if edge == cuts[0] or edge - prev >= CHUNK_COLS or edge == in_cols:
            load_plan.append(("in", prev, edge))
            prev = edge
    assert prev == in_cols

    return {
        "N": N, "F": F, "OUT_F": OUT_F, "BS": BS,
        "wslots": dict(wslots),
        "xt_tiles": xt_tiles,
        "in_blocks": in_blocks, "in_cols": in_cols,
        "segments": segments, "out_cols": out_cols,
        "rows_per_core": rows_per_core, "rt_count": rt_count,
        "load_plan": load_plan,
    }


def _build_nc(meta):
    """Emit the Bass/Tile module for a schedule (value-independent)."""
    Nc = meta["rows_per_core"]
    INC = meta["in_cols"]
    OUTC = meta["out_cols"]
    rt_count = meta["rt_count"]

    nc = bacc.Bacc("TRN2", target_bir_lowering=False, debug=False)
    in_d = nc.dram_tensor("inp", [P, INC], DT_IN, kind="ExternalInput")
    out_d = nc.dram_tensor("out", [Nc, OUTC], DT_OUT, kind="ExternalOutput")

    n_warm = int(os.environ.get("KWARM", "4"))
    flush_cols = int(os.environ.get("KFLUSH", "800"))

    with tile.TileContext(nc) as tc, ExitStack() as ctx:
        in_pool = ctx.enter_context(tc.tile_pool(name="in", bufs=1))
        warm_pool = ctx.enter_context(tc.tile_pool(name="wm", bufs=1))
        psum_pool = ctx.enter_context(tc.tile_pool(name="ps", bufs=4, space="PSUM"))
        out_pool = ctx.enter_context(tc.tile_pool(name="ot", bufs=1))

        inp = in_pool.tile([P, INC], DT_IN)

        # PE warm-up: dummy matmuls on a memset scratch tile (no DMA deps)
        # run during the input-load head and flip HAM to 8/8 early.
        if n_warm:
            wsb = warm_pool.tile([P, 512], DT_IN)
            nc.gpsimd.memset(wsb[:].bitcast(F32), 0)
            wps = psum_pool.tile([P, 1024], F32, tag="mm")
            for _ in range(n_warm):
                nc.tensor.matmul(wps[:, :512], wsb[:, :P], wsb[:, :512],
                                 start=True, stop=True)

        # sequential input stream in consumption order
        for (_, a, b) in meta["load_plan"]:
            nc.sync.dma_start(out=inp[:, a:b], in_=in_d[:, a:b])

        # segment-outer / row-tile-inner: each segment's data is consumed
        # for all row tiles right after it lands, so the PE runs dense and
        # stays ahead of the input stream.
        out_sbs = [out_pool.tile([P, OUTC], DT_OUT, name=f"osb{r}", tag=f"osb{r}")
                   for r in range(rt_count)]
        flushed = [0] * rt_count
        ev = 0
        for si, seg in enumerate(meta["segments"]):
            L = seg["n_obs"] * meta["BS"]
            last = si == len(meta["segments"]) - 1
            for rt in range(rt_count):
                psum = psum_pool.tile([P, 1024], F32, tag="mm")
                for (c0, c1, lc, rb, kr, wc, start, stop) in seg["tasks"]:
                    lhsT = inp[rb:rb + kr, lc + rt * P: lc + (rt + 1) * P]
                    nc.tensor.matmul(
                        psum[:, c0:c1], lhsT, inp[rb:rb + kr, wc:wc + (c1 - c0)],
                        start=start, stop=stop)
                dst = out_sbs[rt][:, seg["out_base"]:seg["out_base"] + L]
                if ev % 2 == 0:
                    nc.scalar.copy(dst, psum[:, :L])
                else:
                    nc.vector.tensor_copy(out=dst, in_=psum[:, :L])
                ev += 1
                done = seg["out_base"] + L
                if done - flushed[rt] >= flush_cols or last:
                    nc.sync.dma_start(
                        out=out_d[rt * P:(rt + 1) * P, flushed[rt]:done],
                        in_=out_sbs[rt][:, flushed[rt]:done])
                    flushed[rt] = done
    nc.compile()
    return nc


def _host_tensors(meta, x2, weight):
    """Build per-core combined input arrays (values only)."""
    BS = meta["BS"]
    Nc = meta["rows_per_core"]
    Ntot = Nc * N_CORES

    if x2.shape[0] < Ntot:
        x2 = np.concatenate(
            [x2, np.zeros((Ntot - x2.shape[0], x2.shape[1]), np.float32)], axis=0)

    wsum = {}
    for (ob_ib, ks) in meta["wslots"].items():
        w = weight[ks[0]]
        for k in ks[1:]:
            w = w + weight[k]
        wsum[ob_ib] = np.ascontiguousarray(w, dtype=np.float32)

    # weight part is identical across cores: fill once
    base = np.zeros((P, meta["in_cols"]), np.float32)
    for blk in meta["in_blocks"]:
        if blk[1] != "w":
            continue
        col, _, rb, uibs, seg_obs = blk
        for r, ib in enumerate(uibs):
            row0 = rb + r * 64
            for j, ob in enumerate(seg_obs):
                w = wsum.get((ob, ib))
                if w is not None:
                    base[row0:row0 + 64, col + j * BS: col + (j + 1) * BS] = w

    in_all = []
    for c in range(N_CORES):
        xs = x2[c * Nc:(c + 1) * Nc]           # [Nc, F]
        comb = base.copy()
        for blk in meta["in_blocks"]:
            if blk[1] != "x":
                continue
            col, _, t = blk[0], blk[1], blk[2]
            for (rbase, ib) in meta["xt_tiles"][t]:
                comb[rbase:rbase + 64, col:col + Nc] = \
                    xs[:, ib * BS:(ib + 1) * BS].T
        in_all.append(np.ascontiguousarray(comb.astype(NP_IN)))
    return in_all


def kernel(**inputs):
    global LAST_RESULT
    x = np.asarray(inputs["x"], dtype=np.float32)
    weight = np.asarray(inputs["weight"], dtype=np.float32)
    bias = np.asarray(inputs["bias"], dtype=np.float32)
    out_idx = np.asarray(inputs["out_block_idx"]).astype(np.int64)
    in_idx = np.asarray(inputs["in_block_idx"]).astype(np.int64)

    B, S, F = x.shape
    N = B * S
    BS = weight.shape[1]
    OUT_F = bias.shape[0]
    x2 = np.ascontiguousarray(x.reshape(N, F))

    key = (N, F, OUT_F, BS, out_idx.tobytes(), in_idx.tobytes())
    if key not in _CACHE:
        meta = _build_schedule(N, F, OUT_F, BS, out_idx, in_idx)
        nc = _build_nc(meta)
        _CACHE[key] = (nc, meta)
    nc, meta = _CACHE[key]

    in_all = _host_tensors(meta, x2, weight)
    in_maps = [{"inp": in_all[c]} for c in range(N_CORES)]
    res = bass_utils.run_bass_kernel_spmd(nc, in_maps, core_ids=list(range(N_CORES)))
    LAST_RESULT = res

    Nc = meta["rows_per_core"]
    dev = np.concatenate(
        [np.asarray(res.results[c]["out"]).astype(np.float32)
         for c in range(N_CORES)], axis=0)
    dev = dev[:N]  # drop row padding

    out = np.zeros((N, OUT_F), np.float32)
    for seg in meta["segments"]:
        b = seg["out_base"]
        for j, ob in enumerate(seg["obs"]):
            out[:, ob * BS:(ob + 1) * BS] = dev[:, b + j * BS: b + (j + 1) * BS]
    if bias.any():
        out += bias
    return out.reshape(B, S, OUT_F)
